# revision 29
# baseline (speedup 1.0000x reference)
"""Greedy bipartite matching (NMS-style) Bass kernel for TRN2.

Algorithm: iterated locally-dominant matching == sequential greedy matching.
Each round: every alive row finds its argmax over alive cols (first
occurrence, via DVE Max8/MaxIndex), every alive col finds its argmax over
alive rows on a transposed copy; pairs that mutually select each other
(integer key match) are matched and their row+col die.

Staged shrinking keeps the DVE scan width proportional to the alive count:
  - rounds 1-2 full-size (actives 512 -> 274 -> 156)
  - compact1: TensorE one-hot gather into a 2-block L1 layout (dense ids
    0..155; rows/cols at block b = id//128, scan windows [256b : 256b+160])
  - 1 L1 round (156 -> <=95)
  - compact2: one-hot gather into a single [128,128] tile
  - 8 cheap tail rounds (<=95 -> <=5)
Matched column ids are recorded per layer (exact under duplicate values)
and translated back to original coordinates at output via the per-layer
id maps (rid/cid one-hot matmuls).  The handful of matrices (~4%) whose
last few rows (<=5) have not converged after 11 rounds are completed
exactly on the host by continuing greedy on the tiny residual submatrix.

Emission is interleaved over groups of G matrices so each engine's static
instruction stream alternates between matrices -- cross-engine round-trips
(PE/ACT/gpsimd broadcast chains) of one matrix overlap with DVE work of the
others.  DMAs issue from the idle SP sequencer (HWDGE) to keep Pool free
for masking and partition broadcasts.
"""

import numpy as np
import concourse.bass as bass
import concourse.bacc as bacc
import concourse.mybir as mybir
from concourse.tile import TileContext
from concourse import library_config

FP = mybir.dt.float32
U32 = mybir.dt.uint32
AL = mybir.AluOpType
AX = mybir.AxisListType

# ---- const layout (free-dim offsets into the [128, CONST_W] consts tensor)
OFF_I128 = 0        # [128,128] identity
OFF_ONESB = 128     # [128,512] ones
OFF_IOTADESC = 640  # [128,512] value 512-j
OFF_UT128 = 1152    # [128,128] upper-tri (q<=p)
OFF_IOTAF128 = 1280  # [128,128] value f
OFF_ROWKEY = 1408   # [128,4] (128k+p)*512
OFF_COLID = 1412    # [128,4] 128k+p
OFF_ROWKEYC = 1416  # [128,1] p*128
OFF_IOTAP = 1417    # [128,1] p
OFF_IOTAF160 = 1418  # [128,160] value f
OFF_RK256 = 1578    # [128,2] (128b+p)*256
CONST_W = 1580


def make_consts() -> np.ndarray:
    c = np.zeros((128, CONST_W), dtype=np.float32)
    c[:, OFF_I128:OFF_I128 + 128] = np.eye(128, dtype=np.float32)
    c[:, OFF_ONESB:OFF_ONESB + 512] = 1.0
    c[:, OFF_IOTADESC:OFF_IOTADESC + 512] = (512.0 - np.arange(512))[None, :]
    q = np.arange(128)
    c[:, OFF_UT128:OFF_UT128 + 128] = (q[:, None] <= q[None, :]).astype(np.float32)
    c[:, OFF_IOTAF128:OFF_IOTAF128 + 128] = q[None, :]
    for k in range(4):
        c[:, OFF_ROWKEY + k] = (128 * k + q) * 512.0
        c[:, OFF_COLID + k] = 128 * k + q
    c[:, OFF_ROWKEYC] = q * 128.0
    c[:, OFF_IOTAP] = q
    c[:, OFF_IOTAF160:OFF_IOTAF160 + 160] = np.arange(160)[None, :]
    for b in range(2):
        c[:, OFF_RK256 + b] = (128 * b + q) * 256.0
    return c


def build_nms_kernel(nc: bass.Bass, out_ap, s_ap, consts_ap, n_mat: int,
                     full_rounds: int = 2, l1_rounds: int = 1,
                     tail_rounds: int = 8,
                     group: int = 4, repeat: int = 1):
    with TileContext(nc) as tc:
        with (
            tc.tile_pool(name="consts", bufs=1) as pool_c,
            tc.tile_pool(name="big", bufs=1) as pool_big,
            tc.tile_pool(name="sm", bufs=1) as pool_sm,
            tc.tile_pool(name="vec", bufs=1) as pool_vec,
            tc.tile_pool(name="outp", bufs=1) as pool_out,
            tc.tile_pool(name="ps", bufs=3, space="PSUM") as pool_ps,
            tc.tile_pool(name="psT", bufs=2, space="PSUM") as pool_psT,
            tc.tile_pool(name="psC", bufs=2, space="PSUM") as pool_psC,
        ):
            C = pool_c.tile([128, CONST_W], FP, name="consts", tag="consts")
            nc.sync.dma_start(out=C[:, :], in_=consts_ap[:, :])
            I128 = C[:, OFF_I128:OFF_I128 + 128]
            onesB = C[:, OFF_ONESB:OFF_ONESB + 512]
            iotaDesc = C[:, OFF_IOTADESC:OFF_IOTADESC + 512]
            UT128 = C[:, OFF_UT128:OFF_UT128 + 128]
            iotaF128 = C[:, OFF_IOTAF128:OFF_IOTAF128 + 128]
            iotaRowKey = C[:, OFF_ROWKEY:OFF_ROWKEY + 4]
            iotaColId = C[:, OFF_COLID:OFF_COLID + 4]
            iotaRowKeyC = C[:, OFF_ROWKEYC:OFF_ROWKEYC + 1]
            iotaP = C[:, OFF_IOTAP:OFF_IOTAP + 1]
            iotaF160 = C[:, OFF_IOTAF160:OFF_IOTAF160 + 160]
            iotaRK256 = C[:, OFF_RK256:OFF_RK256 + 2]

            nc.gpsimd.load_library(library_config.proxy)
            # PE observes the consts DMA once up front.
            warm = pool_psT.tile([128, 128], FP, name="warm", tag="pst")
            nc.tensor.transpose(warm[:, :], I128, I128)

            def big(nm, s, w=512, bufs=1):
                return pool_big.tile([128, w], FP, name=f"{nm}{s}",
                                     tag=f"{nm}{s}", bufs=bufs)

            def sm(nm, s, w=128, dt=FP):
                return pool_sm.tile([128, w], dt, name=f"{nm}{s}",
                                    tag=f"{nm}{s}")

            def vec(nm, s, w=4, p=128, dt=FP):
                return pool_vec.tile([p, w], dt, name=f"{nm}{s}",
                                     tag=f"{nm}{s}")

            # ---------------- per-slot persistent state ----------------
            def make_state(s):
                st = {}
                st["W"] = [big(f"W{k}_", s) for k in range(4)]
                st["Wt"] = [big(f"Wt{k}_", s) for k in range(4)]
                st["trash"] = big("trash_", s)
                st["keyB"] = big("keyB_", s, w=1024)
                st["aliveB"] = big("alvB_", s, w=1024)
                st["rowalive"] = vec("ral_", s)
                st["colalive"] = vec("cal_", s)
                st["mc"] = vec("mc_", s)
                st["rowmax"] = vec("rm_", s)
                st["colmax"] = vec("cm_", s)
                st["argc"] = vec("ac_", s)
                st["argr"] = vec("ar_", s)
                st["m8a"] = vec("m8a_", s, 32)
                st["i8a"] = vec("i8a_", s, 32, dt=U32)
                st["m8ta"] = vec("m8ta_", s, 32)
                st["i8ta"] = vec("i8ta_", s, 32, dt=U32)
                st["rk"] = vec("rk_", s)
                st["ck"] = vec("ck_", s)
                st["t1"] = vec("t1_", s)
                st["t2"] = vec("t2_", s)
                st["t3"] = vec("t3_", s)
                st["t4"] = vec("t4_", s)
                st["mrow"] = vec("mrw_", s)
                st["mcol"] = vec("mcl_", s)
                st["keyRow"] = vec("kR_", s, 1024, p=1)
                st["alvRow"] = vec("aR_", s, 1024, p=1)
                # compact-phase tiles
                st["Wc"] = sm("Wc_", s)
                st["WtC"] = sm("WtC_", s)
                st["scrC"] = sm("sC_", s)
                st["scrC2"] = sm("sC2_", s)
                st["keyBC"] = sm("keyBC_", s, 256)
                st["alvBC"] = sm("alvBC_", s, 256)
                st["GrT"] = [sm(f"GrT{k}_", s) for k in range(4)]
                st["GcT"] = [sm(f"GcT{k}_", s) for k in range(4)]
                st["A"] = [sm(f"A{k}_", s) for k in range(4)]
                st["rid"] = vec("rid_", s, 1)
                st["cid"] = vec("cid_", s, 1)
                st["mcRec"] = vec("mcR_", s, 1)
                st["ralC"] = vec("ralC_", s, 1)
                st["calC"] = vec("calC_", s, 1)
                st["rkC"] = vec("rkC_", s, 1)
                st["ckC"] = vec("ckC_", s, 1)
                st["u1"] = vec("u1_", s, 1)
                st["u2"] = vec("u2_", s, 1)
                st["u3"] = vec("u3_", s, 1)
                st["u4"] = vec("u4_", s, 1)
                st["mrC"] = vec("mrC_", s, 1)
                st["mcC"] = vec("mcC_", s, 1)
                st["m8c"] = vec("m8c_", s, 8)
                st["i8c"] = vec("i8c_", s, 8, dt=U32)
                st["m8d"] = vec("m8d_", s, 8)
                st["i8d"] = vec("i8d_", s, 8, dt=U32)
                st["rmC"] = vec("rmC_", s, 1)
                st["cmC"] = vec("cmC_", s, 1)
                st["acC"] = vec("acC_", s, 1)
                st["arC"] = vec("arC_", s, 1)
                st["keyRowC"] = vec("kRC_", s, 256, p=1)
                st["alvRowC"] = vec("aRC_", s, 256, p=1)
                st["cidRow"] = vec("cidR_", s, 128, p=1)
                st["cidB"] = sm("cidB_", s)
                st["scanrow"] = vec("scan_", s, 12, p=1)
                st["scanrow2"] = vec("scan2_", s, 12, p=1)
                # ---- L1 (2-block compact) state: mostly overlays ----
                st["ral1"] = vec("ral1_", s, 2)
                st["cal1"] = vec("cal1_", s, 2)
                st["mc1"] = vec("mc1_", s, 2)
                st["rid1p"] = vec("rid1p_", s, 2)
                st["cid1p"] = vec("cid1p_", s, 2)
                st["mo1"] = vec("mo1_", s, 2)
                st["g1"] = vec("g1_", s, 2)
                return st

            states = [make_state(s) for s in range(group)]

            def bcast512x2(vec4a, vec4b, rowt, B):
                """two [128,4] -> one [128,1024] (a in cols 0:512, b in 512:1024)."""
                pra = pool_ps.tile([1, 512], FP, name="ps", tag="ps")
                for k in range(4):
                    nc.tensor.matmul(pra[0:1, 128 * k:128 * (k + 1)],
                                     vec4a[:, k:k + 1], I128,
                                     start=True, stop=True)
                nc.scalar.copy(rowt[0:1, 0:512], pra[0:1, :])
                prb = pool_ps.tile([1, 512], FP, name="ps", tag="ps")
                for k in range(4):
                    nc.tensor.matmul(prb[0:1, 128 * k:128 * (k + 1)],
                                     vec4b[:, k:k + 1], I128,
                                     start=True, stop=True)
                nc.scalar.copy(rowt[0:1, 512:1024], prb[0:1, :])
                nc.gpsimd.partition_broadcast(B[:, :], rowt[0:1, :])

            def bcast128(keyc, rowt, B):
                pr = pool_ps.tile([1, 128], FP, name="ps", tag="ps")
                nc.tensor.matmul(pr[0:1, :], keyc[:, 0:1], I128,
                                 start=True, stop=True)
                nc.scalar.copy(rowt[0:1, :], pr[0:1, :])
                nc.gpsimd.partition_broadcast(B[:, :], rowt[0:1, :])

            def bcast128x2(veca, vecb, rowt, B):
                pr = pool_ps.tile([1, 256], FP, name="ps", tag="ps")
                nc.tensor.matmul(pr[0:1, 0:128], veca[:, 0:1], I128,
                                 start=True, stop=True)
                nc.tensor.matmul(pr[0:1, 128:256], vecb[:, 0:1], I128,
                                 start=True, stop=True)
                nc.scalar.copy(rowt[0:1, :], pr[0:1, :])
                nc.gpsimd.partition_broadcast(B[:, :], rowt[0:1, :])

            # ================= stages =================
            def load(st, m):
                for k in range(4):
                    nc.sync.dma_start(out=st["W"][k][:, :],
                                      in_=s_ap[m, 128 * k:128 * (k + 1), :])
                for k in range(4):
                    for r in range(4):
                        pt = pool_psT.tile([128, 128], FP, name="pst", tag="pst")
                        nc.tensor.transpose(pt[:, :],
                                            st["W"][k][:, 128 * r:128 * (r + 1)],
                                            I128)
                        nc.scalar.copy(
                            st["Wt"][r][:, 128 * k:128 * (k + 1)], pt[:, :])
                nc.vector.memset(st["rowalive"][:, :], 1.0)
                nc.vector.memset(st["colalive"][:, :], 1.0)
                nc.vector.memset(st["mc"][:, :], 0.0)

            def full_round_h1(st, r):
                W, Wt = st["W"], st["Wt"]
                m8a, i8a = st["m8a"], st["i8a"]
                m8ta, i8ta = st["m8ta"], st["i8ta"]
                rowmax, colmax = st["rowmax"], st["colmax"]
                argc, argr = st["argc"], st["argr"]
                if r > 0:
                    # Wt-side masking on gpsimd (frees DVE), W-side on DVE
                    for k in range(4):
                        nc.gpsimd.tensor_tensor(out=Wt[k][:, :], in0=Wt[k][:, :],
                                                in1=st["aliveB"][:, 512:1024],
                                                op=AL.mult)
                    for k in range(4):
                        nc.gpsimd.tensor_tensor(out=W[k][:, :], in0=W[k][:, :],
                                                in1=st["aliveB"][:, 0:512],
                                                op=AL.mult)
                for k in range(4):
                    nc.vector.max(m8ta[:, 8 * k:8 * (k + 1)], Wt[k][:, :])
                    nc.vector.max_index(i8ta[:, 8 * k:8 * (k + 1)],
                                        m8ta[:, 8 * k:8 * (k + 1)], Wt[k][:, :])
                nc.vector.tensor_copy(colmax[:, :], m8ta[:, 0:32:8])
                nc.vector.tensor_copy(argr[:, :], i8ta[:, 0:32:8])
                for k in range(4):
                    nc.vector.max(m8a[:, 8 * k:8 * (k + 1)], W[k][:, :])
                    nc.vector.max_index(i8a[:, 8 * k:8 * (k + 1)],
                                        m8a[:, 8 * k:8 * (k + 1)], W[k][:, :])
                nc.vector.tensor_copy(rowmax[:, :], m8a[:, 0:32:8])
                nc.vector.tensor_copy(argc[:, :], i8a[:, 0:32:8])
                rk, ck = st["rk"], st["ck"]
                t1, t2, t3, t4 = st["t1"], st["t2"], st["t3"], st["t4"]
                # ck = (argr*512 + j + 2) * aliveEffC  (col side ready first)
                nc.vector.tensor_scalar(out=t3[:, :], in0=argr[:, :],
                                        scalar1=512.0, scalar2=2.0,
                                        op0=AL.mult, op1=AL.add)
                nc.vector.tensor_tensor(out=t3[:, :], in0=t3[:, :],
                                        in1=iotaColId, op=AL.add)
                nc.vector.scalar_tensor_tensor(out=t4[:, :], in0=colmax[:, :],
                                               scalar=0.0,
                                               in1=st["colalive"][:, :],
                                               op0=AL.is_gt, op1=AL.mult)
                nc.vector.tensor_tensor(out=ck[:, :], in0=t3[:, :],
                                        in1=t4[:, :], op=AL.mult)
                # rk = (i*512 + argc + 2) * aliveEff
                nc.vector.scalar_tensor_tensor(out=t1[:, :], in0=argc[:, :],
                                               scalar=2.0, in1=iotaRowKey,
                                               op0=AL.add, op1=AL.add)
                nc.vector.scalar_tensor_tensor(out=t2[:, :], in0=rowmax[:, :],
                                               scalar=0.0,
                                               in1=st["rowalive"][:, :],
                                               op0=AL.is_gt, op1=AL.mult)
                nc.vector.tensor_tensor(out=rk[:, :], in0=t1[:, :],
                                        in1=t2[:, :], op=AL.mult)
                bcast512x2(ck, rk, st["keyRow"], st["keyB"])

            def full_round_h2(st, r):
                trash = st["trash"]
                argc = st["argc"]
                rk, ck = st["rk"], st["ck"]
                rowmax, colmax = st["rowmax"], st["colmax"]
                t1, t2, t3, t4 = st["t1"], st["t2"], st["t3"], st["t4"]
                # recompute aliveEff guards (t2/t4 still hold them)
                ckB = st["keyB"][:, 0:512]
                rkB = st["keyB"][:, 512:1024]
                mrow, mcol = st["mrow"], st["mcol"]
                # column side first: the round-closing bcast consumes colalive
                # before rowalive, so PE can start its slice matmuls earlier.
                for k in range(4):
                    nc.vector.tensor_scalar(
                        out=trash[:, :], in0=rkB,
                        scalar1=ck[:, k:k + 1], scalar2=0.0,
                        op0=AL.is_equal, op1=AL.max,
                        accum_out=mcol[:, k:k + 1])
                nc.vector.tensor_tensor(out=mcol[:, :], in0=mcol[:, :],
                                        in1=t4[:, :], op=AL.mult)
                nc.vector.scalar_tensor_tensor(out=st["colalive"][:, :],
                                               in0=mcol[:, :], scalar=-1.0,
                                               in1=st["colalive"][:, :],
                                               op0=AL.mult, op1=AL.add)
                for k in range(4):
                    nc.vector.tensor_scalar(
                        out=trash[:, :], in0=ckB,
                        scalar1=rk[:, k:k + 1], scalar2=0.0,
                        op0=AL.is_equal, op1=AL.max,
                        accum_out=mrow[:, k:k + 1])
                nc.vector.tensor_tensor(out=mrow[:, :], in0=mrow[:, :],
                                        in1=t2[:, :], op=AL.mult)
                nc.vector.scalar_tensor_tensor(out=st["rowalive"][:, :],
                                               in0=mrow[:, :], scalar=-1.0,
                                               in1=st["rowalive"][:, :],
                                               op0=AL.mult, op1=AL.add)
                # mc update: matched column index + 1
                nc.vector.tensor_scalar(out=t1[:, :], in0=argc[:, :],
                                        scalar1=1.0, scalar2=None, op0=AL.add)
                nc.vector.tensor_tensor(out=t1[:, :], in0=t1[:, :],
                                        in1=mrow[:, :], op=AL.mult)
                nc.vector.tensor_tensor(out=st["mc"][:, :], in0=st["mc"][:, :],
                                        in1=t1[:, :], op=AL.max)
                if r + 1 < full_rounds:
                    bcast512x2(st["colalive"], st["rowalive"], st["alvRow"],
                               st["aliveB"])

            def block_offsets(alive4, tot, w=4):
                ptot = pool_ps.tile([1, w], FP, name="ps", tag="ps")
                nc.tensor.matmul(ptot[0:1, :], onesB[:, 0:1], alive4[:, :],
                                 start=True, stop=True)
                nc.vector.tensor_copy(tot[0:1, 0:w], ptot[0:1, :])
                nc.vector.tensor_tensor_scan(
                    out=tot[0:1, 4:4 + w], data0=tot[0:1, 0:w],
                    data1=tot[0:1, 0:w],
                    initial=0.0, op0=AL.add, op1=AL.bypass)
                nc.vector.tensor_tensor(out=tot[0:1, 8:8 + w],
                                        in0=tot[0:1, 4:4 + w],
                                        in1=tot[0:1, 0:w], op=AL.subtract)
                pb = pool_ps.tile([128, w], FP, name="ps", tag="ps")
                nc.tensor.matmul(pb[:, :], onesB[0:1, 0:128],
                                 tot[0:1, 8:8 + w], start=True, stop=True)
                return pb

            # ---------- compact 512-space -> L1 2-block space ----------
            # L1 layout: row/col ids dense 0..155; block b = id//128.
            # W1 = W[0]: row-block windows [256b : 256b+160] over col ids.
            # Wt1 = Wt[0]: col-block windows [256b : 256b+160] over row ids.
            def compact1(st):
                ppre = pool_ps.tile([128, 4], FP, name="ps", tag="ps")
                nc.tensor.matmul(ppre[:, :], UT128, st["rowalive"][:, :],
                                 start=True, stop=True)
                posR = st["t1"]
                nc.scalar.copy(posR[:, :], ppre[:, :])
                ppre2 = pool_ps.tile([128, 4], FP, name="ps", tag="ps")
                nc.tensor.matmul(ppre2[:, :], UT128, st["colalive"][:, :],
                                 start=True, stop=True)
                posC = st["t3"]
                nc.scalar.copy(posC[:, :], ppre2[:, :])
                offRB = block_offsets(st["rowalive"], st["scanrow"])
                offCB = block_offsets(st["colalive"], st["scanrow2"])
                nc.vector.tensor_tensor(out=posR[:, :], in0=posR[:, :],
                                        in1=offRB[:, :], op=AL.add)
                nc.vector.tensor_scalar(out=posR[:, :], in0=posR[:, :],
                                        scalar1=-1.0, scalar2=None, op0=AL.add)
                nc.vector.tensor_tensor(out=posC[:, :], in0=posC[:, :],
                                        in1=offCB[:, :], op=AL.add)
                nc.vector.tensor_scalar(out=posC[:, :], in0=posC[:, :],
                                        scalar1=-1.0, scalar2=None, op0=AL.add)
                # posR-128 for dst block 1
                posRm = st["t2"]
                nc.vector.tensor_scalar(out=posRm[:, :], in0=posR[:, :],
                                        scalar1=-128.0, scalar2=None,
                                        op0=AL.add)
                W1, Wt1 = st["W"][0], st["Wt"][0]
                # free-form col one-hots [128,160] (overlay W[2]/W[3])
                GcTf = [st["W"][2][:, 0:160], st["W"][2][:, 160:320],
                        st["W"][2][:, 320:480], st["W"][3][:, 0:160]]
                Asb = [st["W"][1][:, 0:160], st["W"][1][:, 160:320],
                       st["W"][1][:, 320:480], st["W"][3][:, 160:320]]
                cid1B = st["W"][3][:, 320:480]
                for cb in range(4):
                    nc.vector.tensor_scalar(out=GcTf[cb], in0=iotaF160,
                                            scalar1=posC[:, cb:cb + 1],
                                            scalar2=st["colalive"][:, cb:cb + 1],
                                            op0=AL.is_equal, op1=AL.mult)
                # partition-form row one-hots per dst block: b=0 -> GrT, b=1 -> GcT
                for k in range(4):
                    nc.vector.tensor_scalar(out=st["GrT"][k][:, :], in0=iotaF128,
                                            scalar1=posR[:, k:k + 1],
                                            scalar2=st["rowalive"][:, k:k + 1],
                                            op0=AL.is_equal, op1=AL.mult)
                    nc.vector.tensor_scalar(out=st["GcT"][k][:, :], in0=iotaF128,
                                            scalar1=posRm[:, k:k + 1],
                                            scalar2=st["rowalive"][:, k:k + 1],
                                            op0=AL.is_equal, op1=AL.mult)
                # stage A: per src row-block k, gather alive cols -> [128,160]
                for k in range(4):
                    pA = pool_psC.tile([128, 160], FP, name="psA", tag="psA")
                    for cb in range(4):
                        nc.tensor.matmul(pA[:, :],
                                         st["Wt"][cb][:, 128 * k:128 * (k + 1)],
                                         GcTf[cb], start=(cb == 0),
                                         stop=(cb == 3))
                    nc.scalar.copy(Asb[k], pA[:, :])
                # stage B: gather alive rows into 2 dst blocks
                for b in range(2):
                    pB = pool_psC.tile([128, 160], FP, name="psB", tag="psA")
                    for k in range(4):
                        G = st["GrT"][k] if b == 0 else st["GcT"][k]
                        nc.tensor.matmul(pB[:, :], G[:, :], Asb[k],
                                         start=(k == 0), stop=(k == 3))
                    nc.scalar.copy(W1[:, 256 * b:256 * b + 160], pB[:, :])
                # Wt1 via transposes of W1 (full 128-wide copies; pads are 0)
                for bp in range(2):
                    for seg in range(2):
                        pt = pool_psT.tile([128, 128], FP, name="pst",
                                           tag="pst")
                        nc.tensor.transpose(
                            pt[:, :],
                            W1[:, 256 * seg + 128 * bp:256 * seg + 128 * bp + 128],
                            I128)
                        nc.scalar.copy(
                            Wt1[:, 256 * bp + 128 * seg:256 * bp + 128 * seg + 128],
                            pt[:, :])
                # rid1p (orig row id per L1 row slot), cid1row/cid1B/cid1p
                for b in range(2):
                    pr_ = pool_ps.tile([128, 1], FP, name="ps", tag="ps")
                    for k in range(4):
                        G = st["GrT"][k] if b == 0 else st["GcT"][k]
                        nc.tensor.matmul(pr_[:, :], G[:, :],
                                         iotaColId[:, k:k + 1],
                                         start=(k == 0), stop=(k == 3))
                    nc.scalar.copy(st["rid1p"][:, b:b + 1], pr_[:, :])
                pc = pool_psC.tile([1, 160], FP, name="psc1", tag="psA")
                for cb in range(4):
                    nc.tensor.matmul(pc[0:1, :], iotaColId[:, cb:cb + 1],
                                     GcTf[cb], start=(cb == 0), stop=(cb == 3))
                cid1row = st["keyRow"][0:1, 512:672]
                nc.scalar.copy(cid1row, pc[0:1, :])
                nc.gpsimd.partition_broadcast(cid1B, cid1row)
                nc.vector.scalar_tensor_tensor(
                    out=st["trash"][:, 0:128], in0=cid1B[:, 0:128], scalar=0.0,
                    in1=I128, op0=AL.add, op1=AL.mult,
                    accum_out=st["cid1p"][:, 0:1])
                nc.vector.scalar_tensor_tensor(
                    out=st["trash"][:, 128:160], in0=cid1B[:, 128:160],
                    scalar=0.0, in1=I128[:, 0:32], op0=AL.add, op1=AL.mult,
                    accum_out=st["cid1p"][:, 1:2])
                nc.vector.memset(st["ral1"][:, :], 1.0)
                nc.vector.memset(st["cal1"][:, :], 1.0)
                nc.vector.memset(st["mc1"][:, :], 0.0)

            # ---------- L1 rounds (2-block, ids 0..155) ----------
            def l1_h1(st, r):
                W1, Wt1 = st["W"][0], st["Wt"][0]
                aliveB1 = st["aliveB"]
                if r > 0:
                    for b in range(2):
                        nc.gpsimd.tensor_tensor(
                            out=Wt1[:, 256 * b:256 * b + 160],
                            in0=Wt1[:, 256 * b:256 * b + 160],
                            in1=aliveB1[:, 256:416], op=AL.mult)
                    for b in range(2):
                        nc.gpsimd.tensor_tensor(
                            out=W1[:, 256 * b:256 * b + 160],
                            in0=W1[:, 256 * b:256 * b + 160],
                            in1=aliveB1[:, 0:160], op=AL.mult)
                m8t, i8t = st["m8ta"], st["i8ta"]
                m8r, i8r = st["m8a"], st["i8a"]
                for b in range(2):
                    nc.vector.max(m8t[:, 8 * b:8 * (b + 1)],
                                  Wt1[:, 256 * b:256 * b + 160])
                    nc.vector.max_index(i8t[:, 8 * b:8 * (b + 1)],
                                        m8t[:, 8 * b:8 * (b + 1)],
                                        Wt1[:, 256 * b:256 * b + 160])
                nc.vector.tensor_copy(st["colmax"][:, 0:2], m8t[:, 0:16:8])
                nc.vector.tensor_copy(st["argr"][:, 0:2], i8t[:, 0:16:8])
                for b in range(2):
                    nc.vector.max(m8r[:, 8 * b:8 * (b + 1)],
                                  W1[:, 256 * b:256 * b + 160])
                    nc.vector.max_index(i8r[:, 8 * b:8 * (b + 1)],
                                        m8r[:, 8 * b:8 * (b + 1)],
                                        W1[:, 256 * b:256 * b + 160])
                nc.vector.tensor_copy(st["rowmax"][:, 0:2], m8r[:, 0:16:8])
                nc.vector.tensor_copy(st["argc"][:, 0:2], i8r[:, 0:16:8])
                t1, t2, t3, t4 = st["t1"], st["t2"], st["t3"], st["t4"]
                # ck = (argr*256 + colid + 2) * colguard
                nc.vector.tensor_scalar(out=t3[:, 0:2], in0=st["argr"][:, 0:2],
                                        scalar1=256.0, scalar2=2.0,
                                        op0=AL.mult, op1=AL.add)
                nc.vector.tensor_tensor(out=t3[:, 0:2], in0=t3[:, 0:2],
                                        in1=iotaColId[:, 0:2], op=AL.add)
                nc.vector.scalar_tensor_tensor(out=t4[:, 0:2],
                                               in0=st["colmax"][:, 0:2],
                                               scalar=0.0, in1=st["cal1"][:, :],
                                               op0=AL.is_gt, op1=AL.mult)
                nc.vector.tensor_tensor(out=st["ck"][:, 0:2], in0=t3[:, 0:2],
                                        in1=t4[:, 0:2], op=AL.mult)
                # rk = (rowid*256 + argc + 2) * rowguard
                nc.vector.scalar_tensor_tensor(out=t1[:, 0:2],
                                               in0=st["argc"][:, 0:2],
                                               scalar=2.0, in1=iotaRK256,
                                               op0=AL.add, op1=AL.add)
                nc.vector.scalar_tensor_tensor(out=t2[:, 0:2],
                                               in0=st["rowmax"][:, 0:2],
                                               scalar=0.0, in1=st["ral1"][:, :],
                                               op0=AL.is_gt, op1=AL.mult)
                nc.vector.tensor_tensor(out=st["rk"][:, 0:2], in0=t1[:, 0:2],
                                        in1=t2[:, 0:2], op=AL.mult)
                pr = pool_ps.tile([1, 512], FP, name="ps", tag="ps")
                for b in range(2):
                    nc.tensor.matmul(pr[0:1, 128 * b:128 * (b + 1)],
                                     st["ck"][:, b:b + 1], I128,
                                     start=True, stop=True)
                for b in range(2):
                    nc.tensor.matmul(pr[0:1, 256 + 128 * b:256 + 128 * (b + 1)],
                                     st["rk"][:, b:b + 1], I128,
                                     start=True, stop=True)
                nc.scalar.copy(st["keyRow"][0:1, 0:512], pr[0:1, :])
                nc.gpsimd.partition_broadcast(st["keyB"][:, 0:512],
                                              st["keyRow"][0:1, 0:512])

            def l1_h2(st, r, l1_rounds=2):
                keyB1 = st["keyB"]
                trash = st["trash"]
                t2, t4 = st["t2"], st["t4"]
                for b in range(2):
                    nc.vector.tensor_scalar(
                        out=trash[:, 0:160], in0=keyB1[:, 256:416],
                        scalar1=st["ck"][:, b:b + 1], scalar2=0.0,
                        op0=AL.is_equal, op1=AL.max,
                        accum_out=st["mcol"][:, b:b + 1])
                nc.vector.tensor_tensor(out=st["mcol"][:, 0:2],
                                        in0=st["mcol"][:, 0:2],
                                        in1=t4[:, 0:2], op=AL.mult)
                nc.vector.scalar_tensor_tensor(
                    out=st["cal1"][:, :], in0=st["mcol"][:, 0:2], scalar=-1.0,
                    in1=(t4[:, 0:2] if r == 0 else st["cal1"][:, :]),
                    op0=AL.mult, op1=AL.add)
                for b in range(2):
                    nc.vector.tensor_scalar(
                        out=trash[:, 160:320], in0=keyB1[:, 0:160],
                        scalar1=st["rk"][:, b:b + 1], scalar2=0.0,
                        op0=AL.is_equal, op1=AL.max,
                        accum_out=st["mrow"][:, b:b + 1])
                nc.vector.tensor_tensor(out=st["mrow"][:, 0:2],
                                        in0=st["mrow"][:, 0:2],
                                        in1=t2[:, 0:2], op=AL.mult)
                nc.vector.scalar_tensor_tensor(
                    out=st["ral1"][:, :], in0=st["mrow"][:, 0:2], scalar=-1.0,
                    in1=(t2[:, 0:2] if r == 0 else st["ral1"][:, :]),
                    op0=AL.mult, op1=AL.add)
                nc.vector.tensor_scalar(out=st["t1"][:, 0:2],
                                        in0=st["argc"][:, 0:2],
                                        scalar1=1.0, scalar2=None, op0=AL.add)
                nc.vector.tensor_tensor(out=st["t1"][:, 0:2],
                                        in0=st["t1"][:, 0:2],
                                        in1=st["mrow"][:, 0:2], op=AL.mult)
                nc.vector.tensor_tensor(out=st["mc1"][:, :],
                                        in0=st["mc1"][:, :],
                                        in1=st["t1"][:, 0:2], op=AL.max)
                if r + 1 < l1_rounds:
                    pr = pool_ps.tile([1, 512], FP, name="ps", tag="ps")
                    for b in range(2):
                        nc.tensor.matmul(pr[0:1, 128 * b:128 * (b + 1)],
                                         st["cal1"][:, b:b + 1], I128,
                                         start=True, stop=True)
                    for b in range(2):
                        nc.tensor.matmul(
                            pr[0:1, 256 + 128 * b:256 + 128 * (b + 1)],
                            st["ral1"][:, b:b + 1], I128,
                            start=True, stop=True)
                    nc.scalar.copy(st["alvRow"][0:1, 0:512], pr[0:1, :])
                    nc.gpsimd.partition_broadcast(st["aliveB"][:, 0:512],
                                                  st["alvRow"][0:1, 0:512])

            # ---------- compact L1 -> tail [128,128] space ----------
            def compact2(st):
                W1, Wt1 = st["W"][0], st["Wt"][0]
                pp1 = pool_ps.tile([128, 2], FP, name="ps", tag="ps")
                nc.tensor.matmul(pp1[:, :], UT128, st["ral1"][:, :],
                                 start=True, stop=True)
                posR = st["t1"]
                nc.scalar.copy(posR[:, 0:2], pp1[:, :])
                pp2 = pool_ps.tile([128, 2], FP, name="ps", tag="ps")
                nc.tensor.matmul(pp2[:, :], UT128, st["cal1"][:, :],
                                 start=True, stop=True)
                posC = st["t3"]
                nc.scalar.copy(posC[:, 0:2], pp2[:, :])
                offRB = block_offsets(st["ral1"], st["scanrow"], w=2)
                offCB = block_offsets(st["cal1"], st["scanrow2"], w=2)
                nc.vector.tensor_tensor(out=posR[:, 0:2], in0=posR[:, 0:2],
                                        in1=offRB[:, :], op=AL.add)
                nc.vector.tensor_scalar(out=posR[:, 0:2], in0=posR[:, 0:2],
                                        scalar1=-1.0, scalar2=None, op0=AL.add)
                nc.vector.tensor_tensor(out=posC[:, 0:2], in0=posC[:, 0:2],
                                        in1=offCB[:, :], op=AL.add)
                nc.vector.tensor_scalar(out=posC[:, 0:2], in0=posC[:, 0:2],
                                        scalar1=-1.0, scalar2=None, op0=AL.add)
                for b in range(2):
                    nc.vector.tensor_scalar(out=st["GrT"][b][:, :],
                                            in0=iotaF128,
                                            scalar1=posR[:, b:b + 1],
                                            scalar2=st["ral1"][:, b:b + 1],
                                            op0=AL.is_equal, op1=AL.mult)
                    nc.vector.tensor_scalar(out=st["GcT"][b][:, :],
                                            in0=iotaF128,
                                            scalar1=posC[:, b:b + 1],
                                            scalar2=st["cal1"][:, b:b + 1],
                                            op0=AL.is_equal, op1=AL.mult)
                for b in range(2):
                    pA = pool_psT.tile([128, 128], FP, name="pst", tag="pst")
                    for cb in range(2):
                        nc.tensor.matmul(
                            pA[:, :],
                            Wt1[:, 256 * cb + 128 * b:256 * cb + 128 * b + 128],
                            st["GcT"][cb][:, :],
                            start=(cb == 0), stop=(cb == 1))
                    nc.scalar.copy(st["A"][b][:, :], pA[:, :])
                pW = pool_ps.tile([128, 128], FP, name="ps", tag="ps")
                for b in range(2):
                    nc.tensor.matmul(pW[:, :], st["GrT"][b][:, :],
                                     st["A"][b][:, :],
                                     start=(b == 0), stop=(b == 1))
                nc.scalar.copy(st["Wc"][:, :], pW[:, :])
                ptc = pool_ps.tile([128, 128], FP, name="ps", tag="ps")
                nc.tensor.transpose(ptc[:, :], st["Wc"][:, :], I128)
                nc.scalar.copy(st["WtC"][:, :], ptc[:, :])
                prid = pool_ps.tile([128, 1], FP, name="ps", tag="ps")
                for b in range(2):
                    nc.tensor.matmul(prid[:, :], st["GrT"][b][:, :],
                                     st["rid1p"][:, b:b + 1],
                                     start=(b == 0), stop=(b == 1))
                nc.scalar.copy(st["rid"][:, :], prid[:, :])
                pcid = pool_ps.tile([1, 128], FP, name="ps", tag="ps")
                for b in range(2):
                    nc.tensor.matmul(pcid[0:1, :], st["cid1p"][:, b:b + 1],
                                     st["GcT"][b][:, :],
                                     start=(b == 0), stop=(b == 1))
                nc.scalar.copy(st["cidRow"][0:1, :], pcid[0:1, :])
                nc.vector.memset(st["mcRec"][:, :], 0.0)
                nc.vector.memset(st["ralC"][:, :], 1.0)
                nc.vector.memset(st["calC"][:, :], 1.0)

            def tail_round_t1(st, r):
                rmC, cmC = st["rmC"], st["cmC"]
                acC, arC = st["acC"], st["arC"]
                u1, u2, u3, u4 = st["u1"], st["u2"], st["u3"], st["u4"]
                if r > 0:
                    nc.gpsimd.tensor_tensor(out=st["Wc"][:, :],
                                            in0=st["Wc"][:, :],
                                            in1=st["alvBC"][:, 0:128],
                                            op=AL.mult)
                nc.vector.max(st["m8c"][:, :], st["Wc"][:, :])
                nc.vector.max_index(st["i8c"][:, :], st["m8c"][:, :],
                                    st["Wc"][:, :])
                nc.scalar.copy(rmC[:, 0:1], st["m8c"][:, 0:1])
                nc.scalar.copy(acC[:, 0:1], st["i8c"][:, 0:1])
                if r > 0:
                    nc.vector.tensor_tensor(out=st["WtC"][:, :],
                                            in0=st["WtC"][:, :],
                                            in1=st["alvBC"][:, 128:256],
                                            op=AL.mult)
                nc.vector.max(st["m8d"][:, :], st["WtC"][:, :])
                nc.vector.max_index(st["i8d"][:, :], st["m8d"][:, :],
                                    st["WtC"][:, :])
                nc.scalar.copy(cmC[:, 0:1], st["m8d"][:, 0:1])
                nc.scalar.copy(arC[:, 0:1], st["i8d"][:, 0:1])
                rkC, ckC = st["rkC"], st["ckC"]
                nc.vector.scalar_tensor_tensor(out=u1[:, :], in0=acC[:, :],
                                               scalar=2.0, in1=iotaRowKeyC,
                                               op0=AL.add, op1=AL.add)
                nc.vector.scalar_tensor_tensor(out=u2[:, :], in0=rmC[:, :],
                                               scalar=0.0,
                                               in1=st["ralC"][:, :],
                                               op0=AL.is_gt, op1=AL.mult)
                nc.vector.tensor_tensor(out=rkC[:, :], in0=u1[:, :],
                                        in1=u2[:, :], op=AL.mult)
                nc.vector.tensor_scalar(out=u3[:, :], in0=arC[:, :],
                                        scalar1=128.0, scalar2=2.0,
                                        op0=AL.mult, op1=AL.add)
                nc.vector.tensor_tensor(out=u3[:, :], in0=u3[:, :],
                                        in1=iotaP, op=AL.add)
                nc.vector.scalar_tensor_tensor(out=u4[:, :], in0=cmC[:, :],
                                               scalar=0.0,
                                               in1=st["calC"][:, :],
                                               op0=AL.is_gt, op1=AL.mult)
                nc.vector.tensor_tensor(out=ckC[:, :], in0=u3[:, :],
                                        in1=u4[:, :], op=AL.mult)
                bcast128x2(ckC, rkC, st["keyRowC"], st["keyBC"])

            def tail_round_t2(st, r):
                scrC, scrC2 = st["scrC"], st["scrC2"]
                acC = st["acC"]
                # matched-ts dummy outs use scrC/scrC2 (free now)
                rkC, ckC = st["rkC"], st["ckC"]
                u1, u2, u3, u4 = st["u1"], st["u2"], st["u3"], st["u4"]
                mrC, mcC = st["mrC"], st["mcC"]
                nc.vector.tensor_scalar(
                    out=scrC2[:, :], in0=st["keyBC"][:, 0:128],
                    scalar1=rkC[:, 0:1],
                    scalar2=0.0, op0=AL.is_equal, op1=AL.max,
                    accum_out=mrC[:, 0:1])
                nc.vector.tensor_scalar(
                    out=scrC[:, :], in0=st["keyBC"][:, 128:256],
                    scalar1=ckC[:, 0:1],
                    scalar2=0.0, op0=AL.is_equal, op1=AL.max,
                    accum_out=mcC[:, 0:1])
                nc.vector.tensor_tensor(out=mrC[:, :], in0=mrC[:, :],
                                        in1=u2[:, :], op=AL.mult)
                nc.vector.tensor_tensor(out=mcC[:, :], in0=mcC[:, :],
                                        in1=u4[:, :], op=AL.mult)
                nc.vector.tensor_scalar(out=u1[:, :], in0=acC[:, :],
                                        scalar1=1.0, scalar2=None, op0=AL.add)
                nc.vector.tensor_tensor(out=u1[:, :], in0=u1[:, :],
                                        in1=mrC[:, :], op=AL.mult)
                nc.vector.tensor_tensor(out=st["mcRec"][:, :],
                                        in0=st["mcRec"][:, :],
                                        in1=u1[:, :], op=AL.max)
                nc.vector.scalar_tensor_tensor(out=st["ralC"][:, :],
                                               in0=mrC[:, :], scalar=-1.0,
                                               in1=st["ralC"][:, :],
                                               op0=AL.mult, op1=AL.add)
                nc.vector.scalar_tensor_tensor(out=st["calC"][:, :],
                                               in0=mcC[:, :], scalar=-1.0,
                                               in1=st["calC"][:, :],
                                               op0=AL.mult, op1=AL.add)
                if r + 1 < tail_rounds:
                    bcast128x2(st["calC"], st["ralC"], st["alvRowC"],
                               st["alvBC"])

            def output(st, m):
                # orig col of tail matches: onehot(mcRec-1) . cid
                mm1, mo, gt0 = st["u1"], st["u2"], st["u3"]
                nc.vector.tensor_scalar(out=mm1[:, :], in0=st["mcRec"][:, :],
                                        scalar1=-1.0, scalar2=None, op0=AL.add)
                Omc = st["scrC"]
                nc.vector.tensor_scalar(out=Omc[:, :], in0=iotaF128,
                                        scalar1=mm1[:, 0:1], scalar2=None,
                                        op0=AL.is_equal)
                nc.gpsimd.partition_broadcast(st["cidB"][:, :],
                                              st["cidRow"][0:1, :])
                nc.vector.tensor_tensor(out=Omc[:, :], in0=Omc[:, :],
                                        in1=st["cidB"][:, :], op=AL.mult)
                nc.vector.tensor_reduce(out=mo[:, 0:1], in_=Omc[:, :],
                                        axis=AX.X, op=AL.add)
                nc.vector.tensor_scalar(out=gt0[:, :], in0=st["mcRec"][:, :],
                                        scalar1=0.0, scalar2=None, op0=AL.is_gt)
                nc.vector.tensor_scalar(out=mo[:, :], in0=mo[:, :],
                                        scalar1=1.0, scalar2=None, op0=AL.add)
                nc.vector.tensor_tensor(out=mo[:, :], in0=mo[:, :],
                                        in1=gt0[:, :], op=AL.mult)
                pmb = pool_ps.tile([128, 4], FP, name="ps", tag="ps")
                for k in range(4):
                    Gr = st["scrC2"]
                    nc.vector.tensor_scalar(out=st["u4"][:, :],
                                            in0=st["rid"][:, :],
                                            scalar1=float(-128 * k),
                                            scalar2=None, op0=AL.add)
                    nc.vector.tensor_scalar(out=Gr[:, :], in0=iotaF128,
                                            scalar1=st["u4"][:, 0:1],
                                            scalar2=None, op0=AL.is_equal)
                    nc.tensor.matmul(pmb[:, k:k + 1], Gr[:, :], mo[:, 0:1],
                                     start=True, stop=True)
                mcb = st["t2"]
                nc.vector.tensor_copy(mcb[:, :], pmb[:, :])
                nc.vector.tensor_tensor(out=st["mc"][:, :], in0=st["mc"][:, :],
                                        in1=mcb[:, :], op=AL.max)
                # ---- L1 matches: mc1 [128,2] (L1 col id +1) -> orig space
                cid1B = st["W"][3][:, 320:480]
                trash = st["trash"]
                for b in range(2):
                    nc.vector.tensor_scalar(out=st["g1"][:, 0:1],
                                            in0=st["mc1"][:, b:b + 1],
                                            scalar1=-1.0, scalar2=None,
                                            op0=AL.add)
                    nc.vector.tensor_scalar(out=trash[:, 0:160], in0=iotaF160,
                                            scalar1=st["g1"][:, 0:1],
                                            scalar2=None, op0=AL.is_equal)
                    nc.vector.tensor_tensor(out=trash[:, 256:416],
                                            in0=trash[:, 0:160],
                                            in1=cid1B, op=AL.mult)
                    nc.vector.tensor_reduce(out=st["mo1"][:, b:b + 1],
                                            in_=trash[:, 256:416],
                                            axis=AX.X, op=AL.add)
                    nc.vector.tensor_scalar(out=st["g1"][:, 1:2],
                                            in0=st["mc1"][:, b:b + 1],
                                            scalar1=0.0, scalar2=None,
                                            op0=AL.is_gt)
                    nc.vector.tensor_scalar(out=st["mo1"][:, b:b + 1],
                                            in0=st["mo1"][:, b:b + 1],
                                            scalar1=1.0, scalar2=None,
                                            op0=AL.add)
                    nc.vector.tensor_tensor(out=st["mo1"][:, b:b + 1],
                                            in0=st["mo1"][:, b:b + 1],
                                            in1=st["g1"][:, 1:2], op=AL.mult)
                pm1 = pool_ps.tile([128, 4], FP, name="ps", tag="ps")
                for k in range(4):
                    for b in range(2):
                        win = trash[:, 0:128] if b == 0 else trash[:, 128:256]
                        nc.vector.tensor_scalar(out=st["u4"][:, :],
                                                in0=st["rid1p"][:, b:b + 1],
                                                scalar1=float(-128 * k),
                                                scalar2=None, op0=AL.add)
                        nc.vector.tensor_scalar(out=win, in0=iotaF128,
                                                scalar1=st["u4"][:, 0:1],
                                                scalar2=None, op0=AL.is_equal)
                        nc.tensor.matmul(pm1[:, k:k + 1], win,
                                         st["mo1"][:, b:b + 1],
                                         start=(b == 0), stop=(b == 1))
                nc.vector.tensor_copy(mcb[:, :], pm1[:, :])
                nc.vector.tensor_tensor(out=st["mc"][:, :], in0=st["mc"][:, :],
                                        in1=mcb[:, :], op=AL.max)
                s4 = st["t4"]
                nc.vector.tensor_scalar(out=s4[:, :], in0=st["mc"][:, :],
                                        scalar1=-1.0, scalar2=513.0,
                                        op0=AL.mult, op1=AL.add)
                for k in range(4):
                    ot = pool_out.tile([128, 512], FP, name=f"ot{k % 2}",
                                       tag=f"ot{k % 2}")
                    nc.vector.tensor_scalar(out=ot[:, :], in0=iotaDesc,
                                            scalar1=s4[:, k:k + 1],
                                            scalar2=None, op0=AL.is_equal)
                    nc.sync.dma_start(out=out_ap[m, 128 * k:128 * (k + 1), :],
                                      in_=ot[:, :])

            # ================= interleaved emission =================
            mat_list = list(range(n_mat)) * repeat
            for g0 in range(0, len(mat_list), group):
                G = min(group, len(mat_list) - g0)
                for s in range(G):
                    load(states[s], mat_list[g0 + s])
                for r in range(full_rounds):
                    for s in range(G):
                        full_round_h1(states[s], r)
                    for s in range(G):
                        full_round_h2(states[s], r)
                for s in range(G):
                    compact1(states[s])
                for r in range(l1_rounds):
                    for s in range(G):
                        l1_h1(states[s], r)
                    for s in range(G):
                        l1_h2(states[s], r, l1_rounds)
                for s in range(G):
                    compact2(states[s])
                for r in range(tail_rounds):
                    for s in range(G):
                        tail_round_t1(states[s], r)
                    for s in range(G):
                        tail_round_t2(states[s], r)
                for s in range(G):
                    output(states[s], mat_list[g0 + s])
    return nc



# ----------------------------------------------------------------------------
# Host-side entry point: shard the 256-matrix batch over 8 NeuronCores
# (pure data parallelism, 32 matrices per core), run the SPMD kernel,
# reassemble, and exactly recompute any matrix whose output fails the
# permutation sum check (defence in depth; does not trigger on the
# reference input -- tie-breaking on device matches jnp.argmax exactly).
# ----------------------------------------------------------------------------
from concourse.bass_utils import run_bass_kernel_spmd

N_CORES = 8
B, N = 256, 512
MPC = B // N_CORES  # matrices per core


def _greedy_ref_one(w):
    """Exact numpy mirror of the jax reference for one [N,N] matrix."""
    w = w.copy()
    perm = np.zeros_like(w)
    for _ in range(N):
        flat = np.argmax(w)
        r, c = flat // N, flat % N
        perm[r, c] = 1.0
        w[r, :] = 0.0
        w[:, c] = 0.0
    return perm


_CACHE = {}


def _get_graph():
    if "nc" not in _CACHE:
        nc = bacc.Bacc()
        s_ext = nc.declare_dram_parameter("s", [MPC, N, N], FP, isOutput=False)
        c_ext = nc.declare_dram_parameter("consts", [128, CONST_W], FP,
                                          isOutput=False)
        o_ext = nc.declare_dram_parameter("out", [MPC, N, N], FP, isOutput=True)
        build_nms_kernel(nc, o_ext, s_ext, c_ext, n_mat=MPC)
        nc.finalize()
        _CACHE["nc"] = nc
    return _CACHE["nc"]


def kernel(s: np.ndarray) -> np.ndarray:
    s = np.ascontiguousarray(np.asarray(s), dtype=np.float32)
    assert s.shape == (B, N, N)
    nc = _get_graph()
    consts = make_consts()
    shards = s.reshape(N_CORES, MPC, N, N)
    in_maps = [{"s": shards[i], "consts": consts} for i in range(N_CORES)]
    res = run_bass_kernel_spmd(nc, in_maps, core_ids=list(range(N_CORES)))
    out = np.concatenate([np.asarray(res.results[i]["out"])
                          for i in range(N_CORES)], axis=0)
    out = out.reshape(B, N, N).astype(np.float32)
    # safety net: matrices the on-device rounds did not fully converge on
    # (a handful of stragglers with <=5 rows left) are completed exactly by
    # continuing the greedy matching on the tiny residual submatrix; any
    # structurally corrupt perm falls back to a full exact recompute.
    rs = out.sum(axis=2)
    cs = out.sum(axis=1)
    check = np.where((rs != 1.0).any(axis=1) | (cs != 1.0).any(axis=1))[0]
    for b in check:
        if (rs[b] > 1.0).any() or (cs[b] > 1.0).any():
            out[b] = _greedy_ref_one(s[b])
            continue
        ur = np.where(rs[b] == 0.0)[0]
        uc = np.where(cs[b] == 0.0)[0]
        if len(ur) != len(uc):
            out[b] = _greedy_ref_one(s[b])
            continue
        # continue greedy on the residual submatrix (exact: matched pairs of
        # locally-dominant rounds are a prefix-closed subset of greedy's)
        sub = s[b][np.ix_(ur, uc)].copy()
        n = len(ur)
        for _ in range(n):
            flat = np.argmax(sub)
            r, c = flat // n, flat % n
            out[b, ur[r], uc[c]] = 1.0
            sub[r, :] = 0.0
            sub[:, c] = 0.0
    return out



# revision 37
# speedup vs baseline: 1.0307x; 1.0307x over previous
"""Greedy bipartite matching (NMS-style) Bass kernel for TRN2.

Algorithm: iterated locally-dominant matching == sequential greedy matching.
Each round: every alive row finds its argmax over alive cols (first
occurrence, via DVE Max8/MaxIndex), every alive col finds its argmax over
alive rows on a transposed copy; pairs that mutually select each other
(integer key match) are matched and their row+col die.

Staged shrinking keeps the DVE scan width proportional to the alive count:
  - rounds 1-2 full-size (actives 512 -> 274 -> 156)
  - compact1: TensorE one-hot gather into a 2-block L1 layout (dense ids
    0..155; rows/cols at block b = id//128, scan windows [256b : 256b+160])
  - 1 L1 round (156 -> <=95)
  - compact2: one-hot gather into a single [128,128] tile
  - 8 cheap tail rounds (<=95 -> <=5)
Matched column ids are recorded per layer (exact under duplicate values)
and translated back to original coordinates at output via the per-layer
id maps (rid/cid one-hot matmuls).  The handful of matrices (~4%) whose
last few rows (<=5) have not converged after 11 rounds are completed
exactly on the host by continuing greedy on the tiny residual submatrix.

Emission is interleaved over groups of G matrices so each engine's static
instruction stream alternates between matrices -- cross-engine round-trips
(PE/ACT/gpsimd broadcast chains) of one matrix overlap with DVE work of the
others.  DMAs issue from the idle SP sequencer (HWDGE) to keep Pool free
for masking and partition broadcasts.
"""

import numpy as np
import concourse.bass as bass
import concourse.bacc as bacc
import concourse.mybir as mybir
from concourse.tile import TileContext
from concourse import library_config

FP = mybir.dt.float32
U32 = mybir.dt.uint32
AL = mybir.AluOpType
AX = mybir.AxisListType

# ---- const layout (free-dim offsets into the [128, CONST_W] consts tensor)
OFF_I128 = 0        # [128,128] identity
OFF_ONESB = 128     # [128,512] ones
OFF_IOTADESC = 640  # [128,512] value 512-j
OFF_UT128 = 1152    # [128,128] upper-tri (q<=p)
OFF_IOTAF128 = 1280  # [128,128] value f
OFF_ROWKEY = 1408   # [128,4] (128k+p)*512
OFF_COLID = 1412    # [128,4] 128k+p
OFF_ROWKEYC = 1416  # [128,1] p*128
OFF_IOTAP = 1417    # [128,1] p
OFF_IOTAF160 = 1418  # [128,160] value f
OFF_RK256 = 1578    # [128,2] (128b+p)*256
CONST_W = 1580


def make_consts() -> np.ndarray:
    c = np.zeros((128, CONST_W), dtype=np.float32)
    c[:, OFF_I128:OFF_I128 + 128] = np.eye(128, dtype=np.float32)
    c[:, OFF_ONESB:OFF_ONESB + 512] = 1.0
    c[:, OFF_IOTADESC:OFF_IOTADESC + 512] = (512.0 - np.arange(512))[None, :]
    q = np.arange(128)
    c[:, OFF_UT128:OFF_UT128 + 128] = (q[:, None] <= q[None, :]).astype(np.float32)
    c[:, OFF_IOTAF128:OFF_IOTAF128 + 128] = q[None, :]
    for k in range(4):
        c[:, OFF_ROWKEY + k] = (128 * k + q) * 512.0
        c[:, OFF_COLID + k] = 128 * k + q
    c[:, OFF_ROWKEYC] = q * 128.0
    c[:, OFF_IOTAP] = q
    c[:, OFF_IOTAF160:OFF_IOTAF160 + 160] = np.arange(160)[None, :]
    for b in range(2):
        c[:, OFF_RK256 + b] = (128 * b + q) * 256.0
    return c


def build_nms_kernel(nc: bass.Bass, out_ap, s_ap, consts_ap, n_mat: int,
                     full_rounds: int = 2, l1_rounds: int = 1,
                     tail_rounds: int = 8,
                     group: int = 4, repeat: int = 1):
    with TileContext(nc) as tc:
        with (
            tc.tile_pool(name="consts", bufs=1) as pool_c,
            tc.tile_pool(name="big", bufs=1) as pool_big,
            tc.tile_pool(name="sm", bufs=1) as pool_sm,
            tc.tile_pool(name="vec", bufs=1) as pool_vec,
            tc.tile_pool(name="outp", bufs=1) as pool_out,
            tc.tile_pool(name="ps", bufs=3, space="PSUM") as pool_ps,
            tc.tile_pool(name="psT", bufs=2, space="PSUM") as pool_psT,
            tc.tile_pool(name="psC", bufs=3, space="PSUM") as pool_psC,
        ):
            C = pool_c.tile([128, CONST_W], FP, name="consts", tag="consts")
            nc.sync.dma_start(out=C[:, :], in_=consts_ap[:, :])
            I128 = C[:, OFF_I128:OFF_I128 + 128]
            onesB = C[:, OFF_ONESB:OFF_ONESB + 512]
            iotaDesc = C[:, OFF_IOTADESC:OFF_IOTADESC + 512]
            UT128 = C[:, OFF_UT128:OFF_UT128 + 128]
            iotaF128 = C[:, OFF_IOTAF128:OFF_IOTAF128 + 128]
            iotaRowKey = C[:, OFF_ROWKEY:OFF_ROWKEY + 4]
            iotaColId = C[:, OFF_COLID:OFF_COLID + 4]
            iotaRowKeyC = C[:, OFF_ROWKEYC:OFF_ROWKEYC + 1]
            iotaP = C[:, OFF_IOTAP:OFF_IOTAP + 1]
            iotaF160 = C[:, OFF_IOTAF160:OFF_IOTAF160 + 160]
            iotaRK256 = C[:, OFF_RK256:OFF_RK256 + 2]

            nc.gpsimd.load_library(library_config.proxy)
            # PE observes the consts DMA once up front.
            warm = pool_psT.tile([128, 128], FP, name="warm", tag="pst")
            nc.tensor.transpose(warm[:, :], I128, I128)

            def big(nm, s, w=512, bufs=1):
                return pool_big.tile([128, w], FP, name=f"{nm}{s}",
                                     tag=f"{nm}{s}", bufs=bufs)

            def sm(nm, s, w=128, dt=FP):
                return pool_sm.tile([128, w], dt, name=f"{nm}{s}",
                                    tag=f"{nm}{s}")

            def vec(nm, s, w=4, p=128, dt=FP):
                return pool_vec.tile([p, w], dt, name=f"{nm}{s}",
                                     tag=f"{nm}{s}")

            # ---------------- per-slot persistent state ----------------
            trash_shared = pool_big.tile([128, 512], FP, name="trashG",
                                         tag="trashG")

            def make_state(s):
                st = {}
                st["W"] = [big(f"W{k}_", s) for k in range(4)]
                st["Wt"] = [big(f"Wt{k}_", s) for k in range(4)]
                st["trash"] = trash_shared
                st["keyB"] = big("keyB_", s, w=1024)
                st["aliveB"] = big("alvB_", s, w=1024)
                st["rowalive"] = vec("ral_", s)
                st["colalive"] = vec("cal_", s)
                st["mc"] = vec("mc_", s)
                st["rowmax"] = vec("rm_", s)
                st["colmax"] = vec("cm_", s)
                st["argc"] = vec("ac_", s)
                st["argr"] = vec("ar_", s)
                st["m8a"] = vec("m8a_", s, 32)
                st["i8a"] = vec("i8a_", s, 32, dt=U32)
                st["m8ta"] = vec("m8ta_", s, 32)
                st["i8ta"] = vec("i8ta_", s, 32, dt=U32)
                st["rk"] = vec("rk_", s)
                st["ck"] = vec("ck_", s)
                st["t1"] = vec("t1_", s)
                st["t2"] = vec("t2_", s)
                st["t3"] = vec("t3_", s)
                st["t4"] = vec("t4_", s)
                st["mrow"] = vec("mrw_", s)
                st["mcol"] = vec("mcl_", s)
                st["keyRow"] = vec("kR_", s, 1024, p=1)
                st["alvRow"] = vec("aR_", s, 1024, p=1)
                # compact-phase tiles
                st["Wc"] = sm("Wc_", s)
                st["WtC"] = sm("WtC_", s)
                st["scrC"] = sm("sC_", s)
                st["scrC2"] = sm("sC2_", s)
                st["keyBC"] = sm("keyBC_", s, 256)
                st["alvBC"] = sm("alvBC_", s, 256)
                st["GrT"] = [sm(f"GrT{k}_", s) for k in range(4)]
                st["GcT"] = [sm(f"GcT{k}_", s) for k in range(4)]
                st["A"] = [sm(f"A{k}_", s) for k in range(4)]
                st["rid"] = vec("rid_", s, 1)
                st["cid"] = vec("cid_", s, 1)
                st["mcRec"] = vec("mcR_", s, 1)
                st["ralC"] = vec("ralC_", s, 1)
                st["calC"] = vec("calC_", s, 1)
                st["rkC"] = vec("rkC_", s, 1)
                st["ckC"] = vec("ckC_", s, 1)
                st["u1"] = vec("u1_", s, 1)
                st["u2"] = vec("u2_", s, 1)
                st["u3"] = vec("u3_", s, 1)
                st["u4"] = vec("u4_", s, 1)
                st["mrC"] = vec("mrC_", s, 1)
                st["mcC"] = vec("mcC_", s, 1)
                st["m8c"] = vec("m8c_", s, 8)
                st["i8c"] = vec("i8c_", s, 8, dt=U32)
                st["m8d"] = vec("m8d_", s, 8)
                st["i8d"] = vec("i8d_", s, 8, dt=U32)
                st["rmC"] = vec("rmC_", s, 1)
                st["cmC"] = vec("cmC_", s, 1)
                st["acC"] = vec("acC_", s, 1)
                st["arC"] = vec("arC_", s, 1)
                st["keyRowC"] = vec("kRC_", s, 256, p=1)
                st["alvRowC"] = vec("aRC_", s, 256, p=1)
                st["cidRow"] = vec("cidR_", s, 128, p=1)
                st["cidB"] = sm("cidB_", s)
                st["scanrow"] = vec("scan_", s, 12, p=1)
                st["scanrow2"] = vec("scan2_", s, 12, p=1)
                # ---- L1 (2-block compact) state: mostly overlays ----
                st["ral1"] = vec("ral1_", s, 2)
                st["cal1"] = vec("cal1_", s, 2)
                st["mc1"] = vec("mc1_", s, 2)
                st["rid1p"] = vec("rid1p_", s, 2)
                st["cid1p"] = vec("cid1p_", s, 2)
                st["mo1"] = vec("mo1_", s, 2)
                st["g1"] = vec("g1_", s, 2)
                st["cid1Bs"] = sm("c1B_", s, 160)
                return st

            states = [make_state(s) for s in range(group)]

            def bcast512x2(vec4a, vec4b, rowt, B):
                """two [128,4] -> one [128,1024] (a in cols 0:512, b in 512:1024)."""
                pra = pool_ps.tile([1, 512], FP, name="ps", tag="ps")
                for k in range(4):
                    nc.tensor.matmul(pra[0:1, 128 * k:128 * (k + 1)],
                                     vec4a[:, k:k + 1], I128,
                                     start=True, stop=True)
                nc.scalar.copy(rowt[0:1, 0:512], pra[0:1, :])
                prb = pool_ps.tile([1, 512], FP, name="ps", tag="ps")
                for k in range(4):
                    nc.tensor.matmul(prb[0:1, 128 * k:128 * (k + 1)],
                                     vec4b[:, k:k + 1], I128,
                                     start=True, stop=True)
                nc.scalar.copy(rowt[0:1, 512:1024], prb[0:1, :])
                nc.gpsimd.partition_broadcast(B[:, :], rowt[0:1, :])

            def bcast128(keyc, rowt, B):
                pr = pool_ps.tile([1, 128], FP, name="ps", tag="ps")
                nc.tensor.matmul(pr[0:1, :], keyc[:, 0:1], I128,
                                 start=True, stop=True)
                nc.scalar.copy(rowt[0:1, :], pr[0:1, :])
                nc.gpsimd.partition_broadcast(B[:, :], rowt[0:1, :])

            def bcast128x2(veca, vecb, rowt, B):
                pr = pool_ps.tile([1, 256], FP, name="ps", tag="ps")
                nc.tensor.matmul(pr[0:1, 0:128], veca[:, 0:1], I128,
                                 start=True, stop=True)
                nc.tensor.matmul(pr[0:1, 128:256], vecb[:, 0:1], I128,
                                 start=True, stop=True)
                nc.scalar.copy(rowt[0:1, :], pr[0:1, :])
                nc.gpsimd.partition_broadcast(B[:, :], rowt[0:1, :])

            # ================= stages =================
            def load(st, m):
                for k in range(4):
                    nc.sync.dma_start(out=st["W"][k][:, :],
                                      in_=s_ap[m, 128 * k:128 * (k + 1), :])
                for k in range(4):
                    for r in range(4):
                        pt = pool_psT.tile([128, 128], FP, name="pst", tag="pst")
                        nc.tensor.transpose(pt[:, :],
                                            st["W"][k][:, 128 * r:128 * (r + 1)],
                                            I128)
                        nc.scalar.copy(
                            st["Wt"][r][:, 128 * k:128 * (k + 1)], pt[:, :])

            def init_vecs(st):
                nc.vector.memset(st["rowalive"][:, :], 1.0)
                nc.vector.memset(st["colalive"][:, :], 1.0)
                nc.vector.memset(st["mc"][:, :], 0.0)

            def full_round_h1(st, r):
                W, Wt = st["W"], st["Wt"]
                m8a, i8a = st["m8a"], st["i8a"]
                m8ta, i8ta = st["m8ta"], st["i8ta"]
                rowmax, colmax = st["rowmax"], st["colmax"]
                argc, argr = st["argc"], st["argr"]
                if r > 0:
                    # Wt-side masking on gpsimd (frees DVE), W-side on DVE
                    for k in range(4):
                        nc.gpsimd.tensor_tensor(out=Wt[k][:, :], in0=Wt[k][:, :],
                                                in1=st["aliveB"][:, 512:1024],
                                                op=AL.mult)
                    for k in range(4):
                        nc.gpsimd.tensor_tensor(out=W[k][:, :], in0=W[k][:, :],
                                                in1=st["aliveB"][:, 0:512],
                                                op=AL.mult)
                for k in range(4):
                    nc.vector.max(m8ta[:, 8 * k:8 * (k + 1)], Wt[k][:, :])
                    nc.vector.max_index(i8ta[:, 8 * k:8 * (k + 1)],
                                        m8ta[:, 8 * k:8 * (k + 1)], Wt[k][:, :])
                nc.vector.tensor_copy(colmax[:, :], m8ta[:, 0:32:8])
                nc.vector.tensor_copy(argr[:, :], i8ta[:, 0:32:8])
                for k in range(4):
                    nc.vector.max(m8a[:, 8 * k:8 * (k + 1)], W[k][:, :])
                    nc.vector.max_index(i8a[:, 8 * k:8 * (k + 1)],
                                        m8a[:, 8 * k:8 * (k + 1)], W[k][:, :])
                nc.vector.tensor_copy(rowmax[:, :], m8a[:, 0:32:8])
                nc.vector.tensor_copy(argc[:, :], i8a[:, 0:32:8])
                rk, ck = st["rk"], st["ck"]
                t1, t2, t3, t4 = st["t1"], st["t2"], st["t3"], st["t4"]
                # ck = (argr*512 + j + 2) * aliveEffC  (col side ready first)
                nc.vector.tensor_scalar(out=t3[:, :], in0=argr[:, :],
                                        scalar1=512.0, scalar2=2.0,
                                        op0=AL.mult, op1=AL.add)
                nc.vector.tensor_tensor(out=t3[:, :], in0=t3[:, :],
                                        in1=iotaColId, op=AL.add)
                nc.vector.scalar_tensor_tensor(out=t4[:, :], in0=colmax[:, :],
                                               scalar=0.0,
                                               in1=st["colalive"][:, :],
                                               op0=AL.is_gt, op1=AL.mult)
                nc.vector.tensor_tensor(out=ck[:, :], in0=t3[:, :],
                                        in1=t4[:, :], op=AL.mult)
                # rk = (i*512 + argc + 2) * aliveEff
                nc.vector.scalar_tensor_tensor(out=t1[:, :], in0=argc[:, :],
                                               scalar=2.0, in1=iotaRowKey,
                                               op0=AL.add, op1=AL.add)
                nc.vector.scalar_tensor_tensor(out=t2[:, :], in0=rowmax[:, :],
                                               scalar=0.0,
                                               in1=st["rowalive"][:, :],
                                               op0=AL.is_gt, op1=AL.mult)
                nc.vector.tensor_tensor(out=rk[:, :], in0=t1[:, :],
                                        in1=t2[:, :], op=AL.mult)
                bcast512x2(ck, rk, st["keyRow"], st["keyB"])

            def full_round_h2(st, r):
                trash = st["trash"]
                argc = st["argc"]
                rk, ck = st["rk"], st["ck"]
                rowmax, colmax = st["rowmax"], st["colmax"]
                t1, t2, t3, t4 = st["t1"], st["t2"], st["t3"], st["t4"]
                # recompute aliveEff guards (t2/t4 still hold them)
                ckB = st["keyB"][:, 0:512]
                rkB = st["keyB"][:, 512:1024]
                mrow, mcol = st["mrow"], st["mcol"]
                # column side first: the round-closing bcast consumes colalive
                # before rowalive, so PE can start its slice matmuls earlier.
                for k in range(4):
                    nc.vector.tensor_scalar(
                        out=trash[:, :], in0=rkB,
                        scalar1=ck[:, k:k + 1], scalar2=0.0,
                        op0=AL.is_equal, op1=AL.max,
                        accum_out=mcol[:, k:k + 1])
                nc.vector.tensor_tensor(out=mcol[:, :], in0=mcol[:, :],
                                        in1=t4[:, :], op=AL.mult)
                nc.vector.scalar_tensor_tensor(out=st["colalive"][:, :],
                                               in0=mcol[:, :], scalar=-1.0,
                                               in1=st["colalive"][:, :],
                                               op0=AL.mult, op1=AL.add)
                for k in range(4):
                    nc.vector.tensor_scalar(
                        out=trash[:, :], in0=ckB,
                        scalar1=rk[:, k:k + 1], scalar2=0.0,
                        op0=AL.is_equal, op1=AL.max,
                        accum_out=mrow[:, k:k + 1])
                nc.vector.tensor_tensor(out=mrow[:, :], in0=mrow[:, :],
                                        in1=t2[:, :], op=AL.mult)
                nc.vector.scalar_tensor_tensor(out=st["rowalive"][:, :],
                                               in0=mrow[:, :], scalar=-1.0,
                                               in1=st["rowalive"][:, :],
                                               op0=AL.mult, op1=AL.add)
                # mc update: matched column index + 1
                nc.vector.tensor_scalar(out=t1[:, :], in0=argc[:, :],
                                        scalar1=1.0, scalar2=None, op0=AL.add)
                nc.vector.tensor_tensor(out=t1[:, :], in0=t1[:, :],
                                        in1=mrow[:, :], op=AL.mult)
                nc.vector.tensor_tensor(out=st["mc"][:, :], in0=st["mc"][:, :],
                                        in1=t1[:, :], op=AL.max)
                if r + 1 < full_rounds:
                    bcast512x2(st["colalive"], st["rowalive"], st["alvRow"],
                               st["aliveB"])

            def block_offsets(alive4, tot, w=4):
                ptot = pool_ps.tile([1, w], FP, name="ps", tag="ps")
                nc.tensor.matmul(ptot[0:1, :], onesB[:, 0:1], alive4[:, :],
                                 start=True, stop=True)
                nc.vector.tensor_copy(tot[0:1, 0:w], ptot[0:1, :])
                nc.vector.tensor_tensor_scan(
                    out=tot[0:1, 4:4 + w], data0=tot[0:1, 0:w],
                    data1=tot[0:1, 0:w],
                    initial=0.0, op0=AL.add, op1=AL.bypass)
                nc.vector.tensor_tensor(out=tot[0:1, 8:8 + w],
                                        in0=tot[0:1, 4:4 + w],
                                        in1=tot[0:1, 0:w], op=AL.subtract)
                pb = pool_ps.tile([128, w], FP, name="ps", tag="ps")
                nc.tensor.matmul(pb[:, :], onesB[0:1, 0:128],
                                 tot[0:1, 8:8 + w], start=True, stop=True)
                return pb

            # ---------- compact 512-space -> L1 2-block space ----------
            # L1 layout: row/col ids dense 0..155; block b = id//128.
            # W1 = W[0]: row-block windows [256b : 256b+160] over col ids.
            # Wt1 = Wt[0]: col-block windows [256b : 256b+160] over row ids.
            def compact1(st):
                ppre = pool_ps.tile([128, 4], FP, name="ps", tag="ps")
                nc.tensor.matmul(ppre[:, :], UT128, st["rowalive"][:, :],
                                 start=True, stop=True)
                posR = st["t1"]
                nc.scalar.copy(posR[:, :], ppre[:, :])
                ppre2 = pool_ps.tile([128, 4], FP, name="ps", tag="ps")
                nc.tensor.matmul(ppre2[:, :], UT128, st["colalive"][:, :],
                                 start=True, stop=True)
                posC = st["t3"]
                nc.scalar.copy(posC[:, :], ppre2[:, :])
                offRB = block_offsets(st["rowalive"], st["scanrow"])
                offCB = block_offsets(st["colalive"], st["scanrow2"])
                nc.vector.tensor_tensor(out=posR[:, :], in0=posR[:, :],
                                        in1=offRB[:, :], op=AL.add)
                nc.vector.tensor_scalar(out=posR[:, :], in0=posR[:, :],
                                        scalar1=-1.0, scalar2=None, op0=AL.add)
                nc.vector.tensor_tensor(out=posC[:, :], in0=posC[:, :],
                                        in1=offCB[:, :], op=AL.add)
                nc.vector.tensor_scalar(out=posC[:, :], in0=posC[:, :],
                                        scalar1=-1.0, scalar2=None, op0=AL.add)
                # posR-128 for dst block 1
                posRm = st["t2"]
                nc.vector.tensor_scalar(out=posRm[:, :], in0=posR[:, :],
                                        scalar1=-128.0, scalar2=None,
                                        op0=AL.add)
                W1, Wt1 = st["W"][0], st["Wt"][0]
                # free-form col one-hots [128,160] (overlay W[2]/W[3])
                GcTf = [st["W"][2][:, 0:160], st["W"][2][:, 160:320],
                        st["W"][2][:, 320:480], st["W"][3][:, 0:160]]
                Asb = [st["W"][1][:, 0:160], st["W"][1][:, 160:320],
                       st["W"][1][:, 320:480], st["W"][3][:, 160:320]]
                cid1B = st["cid1Bs"][:, :]
                for cb in range(4):
                    nc.vector.tensor_scalar(out=GcTf[cb], in0=iotaF160,
                                            scalar1=posC[:, cb:cb + 1],
                                            scalar2=st["colalive"][:, cb:cb + 1],
                                            op0=AL.is_equal, op1=AL.mult)
                # partition-form row one-hots per dst block: b=0 -> GrT, b=1 -> GcT
                for k in range(4):
                    nc.vector.tensor_scalar(out=st["GrT"][k][:, :], in0=iotaF128,
                                            scalar1=posR[:, k:k + 1],
                                            scalar2=st["rowalive"][:, k:k + 1],
                                            op0=AL.is_equal, op1=AL.mult)
                    nc.vector.tensor_scalar(out=st["GcT"][k][:, :], in0=iotaF128,
                                            scalar1=posRm[:, k:k + 1],
                                            scalar2=st["rowalive"][:, k:k + 1],
                                            op0=AL.is_equal, op1=AL.mult)
                # stage A: per src row-block k, gather alive cols -> [128,160]
                for k in range(4):
                    pA = pool_psC.tile([128, 160], FP, name="psA", tag="psA")
                    for cb in range(4):
                        nc.tensor.matmul(pA[:, :],
                                         st["Wt"][cb][:, 128 * k:128 * (k + 1)],
                                         GcTf[cb], start=(cb == 0),
                                         stop=(cb == 3))
                    nc.scalar.copy(Asb[k], pA[:, :])
                # stage B: gather alive rows into 2 dst blocks
                for b in range(2):
                    pB = pool_psC.tile([128, 160], FP, name="psB", tag="psA")
                    for k in range(4):
                        G = st["GrT"][k] if b == 0 else st["GcT"][k]
                        nc.tensor.matmul(pB[:, :], G[:, :], Asb[k],
                                         start=(k == 0), stop=(k == 3))
                    nc.scalar.copy(W1[:, 256 * b:256 * b + 160], pB[:, :])
                # Wt1 via transposes of W1 (full 128-wide copies; pads are 0)
                for bp in range(2):
                    for seg in range(2):
                        pt = pool_psT.tile([128, 128], FP, name="pst",
                                           tag="pst")
                        nc.tensor.transpose(
                            pt[:, :],
                            W1[:, 256 * seg + 128 * bp:256 * seg + 128 * bp + 128],
                            I128)
                        nc.scalar.copy(
                            Wt1[:, 256 * bp + 128 * seg:256 * bp + 128 * seg + 128],
                            pt[:, :])
                # rid1p (orig row id per L1 row slot), cid1row/cid1B/cid1p
                for b in range(2):
                    pr_ = pool_ps.tile([128, 1], FP, name="ps", tag="ps")
                    for k in range(4):
                        G = st["GrT"][k] if b == 0 else st["GcT"][k]
                        nc.tensor.matmul(pr_[:, :], G[:, :],
                                         iotaColId[:, k:k + 1],
                                         start=(k == 0), stop=(k == 3))
                    nc.scalar.copy(st["rid1p"][:, b:b + 1], pr_[:, :])
                pc = pool_psC.tile([1, 160], FP, name="psc1", tag="psA")
                for cb in range(4):
                    nc.tensor.matmul(pc[0:1, :], iotaColId[:, cb:cb + 1],
                                     GcTf[cb], start=(cb == 0), stop=(cb == 3))
                cid1row = st["keyRow"][0:1, 512:672]
                nc.scalar.copy(cid1row, pc[0:1, :])
                nc.gpsimd.partition_broadcast(cid1B, cid1row)
                nc.vector.scalar_tensor_tensor(
                    out=st["trash"][:, 0:128], in0=cid1B[:, 0:128], scalar=0.0,
                    in1=I128, op0=AL.add, op1=AL.mult,
                    accum_out=st["cid1p"][:, 0:1])
                nc.vector.scalar_tensor_tensor(
                    out=st["trash"][:, 128:160], in0=cid1B[:, 128:160],
                    scalar=0.0, in1=I128[:, 0:32], op0=AL.add, op1=AL.mult,
                    accum_out=st["cid1p"][:, 1:2])
                nc.vector.memset(st["ral1"][:, :], 1.0)
                nc.vector.memset(st["cal1"][:, :], 1.0)
                nc.vector.memset(st["mc1"][:, :], 0.0)

            # ---------- L1 rounds (2-block, ids 0..155) ----------
            def l1_h1(st, r):
                W1, Wt1 = st["W"][0], st["Wt"][0]
                aliveB1 = st["aliveB"]
                if r > 0:
                    for b in range(2):
                        nc.gpsimd.tensor_tensor(
                            out=Wt1[:, 256 * b:256 * b + 160],
                            in0=Wt1[:, 256 * b:256 * b + 160],
                            in1=aliveB1[:, 256:416], op=AL.mult)
                    for b in range(2):
                        nc.gpsimd.tensor_tensor(
                            out=W1[:, 256 * b:256 * b + 160],
                            in0=W1[:, 256 * b:256 * b + 160],
                            in1=aliveB1[:, 0:160], op=AL.mult)
                m8t, i8t = st["m8ta"], st["i8ta"]
                m8r, i8r = st["m8a"], st["i8a"]
                for b in range(2):
                    nc.vector.max(m8t[:, 8 * b:8 * (b + 1)],
                                  Wt1[:, 256 * b:256 * b + 160])
                    nc.vector.max_index(i8t[:, 8 * b:8 * (b + 1)],
                                        m8t[:, 8 * b:8 * (b + 1)],
                                        Wt1[:, 256 * b:256 * b + 160])
                nc.vector.tensor_copy(st["colmax"][:, 0:2], m8t[:, 0:16:8])
                nc.vector.tensor_copy(st["argr"][:, 0:2], i8t[:, 0:16:8])
                for b in range(2):
                    nc.vector.max(m8r[:, 8 * b:8 * (b + 1)],
                                  W1[:, 256 * b:256 * b + 160])
                    nc.vector.max_index(i8r[:, 8 * b:8 * (b + 1)],
                                        m8r[:, 8 * b:8 * (b + 1)],
                                        W1[:, 256 * b:256 * b + 160])
                nc.vector.tensor_copy(st["rowmax"][:, 0:2], m8r[:, 0:16:8])
                nc.vector.tensor_copy(st["argc"][:, 0:2], i8r[:, 0:16:8])
                t1, t2, t3, t4 = st["t1"], st["t2"], st["t3"], st["t4"]
                # ck = (argr*256 + colid + 2) * colguard
                nc.vector.tensor_scalar(out=t3[:, 0:2], in0=st["argr"][:, 0:2],
                                        scalar1=256.0, scalar2=2.0,
                                        op0=AL.mult, op1=AL.add)
                nc.vector.tensor_tensor(out=t3[:, 0:2], in0=t3[:, 0:2],
                                        in1=iotaColId[:, 0:2], op=AL.add)
                nc.vector.scalar_tensor_tensor(out=t4[:, 0:2],
                                               in0=st["colmax"][:, 0:2],
                                               scalar=0.0, in1=st["cal1"][:, :],
                                               op0=AL.is_gt, op1=AL.mult)
                nc.vector.tensor_tensor(out=st["ck"][:, 0:2], in0=t3[:, 0:2],
                                        in1=t4[:, 0:2], op=AL.mult)
                # rk = (rowid*256 + argc + 2) * rowguard
                nc.vector.scalar_tensor_tensor(out=t1[:, 0:2],
                                               in0=st["argc"][:, 0:2],
                                               scalar=2.0, in1=iotaRK256,
                                               op0=AL.add, op1=AL.add)
                nc.vector.scalar_tensor_tensor(out=t2[:, 0:2],
                                               in0=st["rowmax"][:, 0:2],
                                               scalar=0.0, in1=st["ral1"][:, :],
                                               op0=AL.is_gt, op1=AL.mult)
                nc.vector.tensor_tensor(out=st["rk"][:, 0:2], in0=t1[:, 0:2],
                                        in1=t2[:, 0:2], op=AL.mult)
                pr = pool_ps.tile([1, 512], FP, name="ps", tag="ps")
                for b in range(2):
                    nc.tensor.matmul(pr[0:1, 128 * b:128 * (b + 1)],
                                     st["ck"][:, b:b + 1], I128,
                                     start=True, stop=True)
                for b in range(2):
                    nc.tensor.matmul(pr[0:1, 256 + 128 * b:256 + 128 * (b + 1)],
                                     st["rk"][:, b:b + 1], I128,
                                     start=True, stop=True)
                nc.scalar.copy(st["keyRow"][0:1, 0:512], pr[0:1, :])
                nc.gpsimd.partition_broadcast(st["keyB"][:, 0:512],
                                              st["keyRow"][0:1, 0:512])

            def l1_h2(st, r, l1_rounds=2):
                keyB1 = st["keyB"]
                trash = st["trash"]
                t2, t4 = st["t2"], st["t4"]
                for b in range(2):
                    nc.vector.tensor_scalar(
                        out=trash[:, 0:160], in0=keyB1[:, 256:416],
                        scalar1=st["ck"][:, b:b + 1], scalar2=0.0,
                        op0=AL.is_equal, op1=AL.max,
                        accum_out=st["mcol"][:, b:b + 1])
                nc.vector.tensor_tensor(out=st["mcol"][:, 0:2],
                                        in0=st["mcol"][:, 0:2],
                                        in1=t4[:, 0:2], op=AL.mult)
                nc.vector.scalar_tensor_tensor(
                    out=st["cal1"][:, :], in0=st["mcol"][:, 0:2], scalar=-1.0,
                    in1=(t4[:, 0:2] if r == 0 else st["cal1"][:, :]),
                    op0=AL.mult, op1=AL.add)
                for b in range(2):
                    nc.vector.tensor_scalar(
                        out=trash[:, 160:320], in0=keyB1[:, 0:160],
                        scalar1=st["rk"][:, b:b + 1], scalar2=0.0,
                        op0=AL.is_equal, op1=AL.max,
                        accum_out=st["mrow"][:, b:b + 1])
                nc.vector.tensor_tensor(out=st["mrow"][:, 0:2],
                                        in0=st["mrow"][:, 0:2],
                                        in1=t2[:, 0:2], op=AL.mult)
                nc.vector.scalar_tensor_tensor(
                    out=st["ral1"][:, :], in0=st["mrow"][:, 0:2], scalar=-1.0,
                    in1=(t2[:, 0:2] if r == 0 else st["ral1"][:, :]),
                    op0=AL.mult, op1=AL.add)
                nc.vector.tensor_scalar(out=st["t1"][:, 0:2],
                                        in0=st["argc"][:, 0:2],
                                        scalar1=1.0, scalar2=None, op0=AL.add)
                nc.vector.tensor_tensor(out=st["t1"][:, 0:2],
                                        in0=st["t1"][:, 0:2],
                                        in1=st["mrow"][:, 0:2], op=AL.mult)
                nc.vector.tensor_tensor(out=st["mc1"][:, :],
                                        in0=st["mc1"][:, :],
                                        in1=st["t1"][:, 0:2], op=AL.max)
                if r + 1 < l1_rounds:
                    pr = pool_ps.tile([1, 512], FP, name="ps", tag="ps")
                    for b in range(2):
                        nc.tensor.matmul(pr[0:1, 128 * b:128 * (b + 1)],
                                         st["cal1"][:, b:b + 1], I128,
                                         start=True, stop=True)
                    for b in range(2):
                        nc.tensor.matmul(
                            pr[0:1, 256 + 128 * b:256 + 128 * (b + 1)],
                            st["ral1"][:, b:b + 1], I128,
                            start=True, stop=True)
                    nc.scalar.copy(st["alvRow"][0:1, 0:512], pr[0:1, :])
                    nc.gpsimd.partition_broadcast(st["aliveB"][:, 0:512],
                                                  st["alvRow"][0:1, 0:512])

            # ---------- compact L1 -> tail [128,128] space ----------
            def compact2(st):
                W1, Wt1 = st["W"][0], st["Wt"][0]
                pp1 = pool_ps.tile([128, 2], FP, name="ps", tag="ps")
                nc.tensor.matmul(pp1[:, :], UT128, st["ral1"][:, :],
                                 start=True, stop=True)
                posR = st["t1"]
                nc.scalar.copy(posR[:, 0:2], pp1[:, :])
                pp2 = pool_ps.tile([128, 2], FP, name="ps", tag="ps")
                nc.tensor.matmul(pp2[:, :], UT128, st["cal1"][:, :],
                                 start=True, stop=True)
                posC = st["t3"]
                nc.scalar.copy(posC[:, 0:2], pp2[:, :])
                offRB = block_offsets(st["ral1"], st["scanrow"], w=2)
                offCB = block_offsets(st["cal1"], st["scanrow2"], w=2)
                nc.vector.tensor_tensor(out=posR[:, 0:2], in0=posR[:, 0:2],
                                        in1=offRB[:, :], op=AL.add)
                nc.vector.tensor_scalar(out=posR[:, 0:2], in0=posR[:, 0:2],
                                        scalar1=-1.0, scalar2=None, op0=AL.add)
                nc.vector.tensor_tensor(out=posC[:, 0:2], in0=posC[:, 0:2],
                                        in1=offCB[:, :], op=AL.add)
                nc.vector.tensor_scalar(out=posC[:, 0:2], in0=posC[:, 0:2],
                                        scalar1=-1.0, scalar2=None, op0=AL.add)
                for b in range(2):
                    nc.vector.tensor_scalar(out=st["GrT"][b][:, :],
                                            in0=iotaF128,
                                            scalar1=posR[:, b:b + 1],
                                            scalar2=st["ral1"][:, b:b + 1],
                                            op0=AL.is_equal, op1=AL.mult)
                    nc.vector.tensor_scalar(out=st["GcT"][b][:, :],
                                            in0=iotaF128,
                                            scalar1=posC[:, b:b + 1],
                                            scalar2=st["cal1"][:, b:b + 1],
                                            op0=AL.is_equal, op1=AL.mult)
                for b in range(2):
                    pA = pool_psT.tile([128, 128], FP, name="pst", tag="pst")
                    for cb in range(2):
                        nc.tensor.matmul(
                            pA[:, :],
                            Wt1[:, 256 * cb + 128 * b:256 * cb + 128 * b + 128],
                            st["GcT"][cb][:, :],
                            start=(cb == 0), stop=(cb == 1))
                    nc.scalar.copy(st["A"][b][:, :], pA[:, :])
                pW = pool_ps.tile([128, 128], FP, name="ps", tag="ps")
                for b in range(2):
                    nc.tensor.matmul(pW[:, :], st["GrT"][b][:, :],
                                     st["A"][b][:, :],
                                     start=(b == 0), stop=(b == 1))
                nc.scalar.copy(st["Wc"][:, :], pW[:, :])
                ptc = pool_ps.tile([128, 128], FP, name="ps", tag="ps")
                nc.tensor.transpose(ptc[:, :], st["Wc"][:, :], I128)
                nc.scalar.copy(st["WtC"][:, :], ptc[:, :])
                prid = pool_ps.tile([128, 1], FP, name="ps", tag="ps")
                for b in range(2):
                    nc.tensor.matmul(prid[:, :], st["GrT"][b][:, :],
                                     st["rid1p"][:, b:b + 1],
                                     start=(b == 0), stop=(b == 1))
                nc.scalar.copy(st["rid"][:, :], prid[:, :])
                pcid = pool_ps.tile([1, 128], FP, name="ps", tag="ps")
                for b in range(2):
                    nc.tensor.matmul(pcid[0:1, :], st["cid1p"][:, b:b + 1],
                                     st["GcT"][b][:, :],
                                     start=(b == 0), stop=(b == 1))
                nc.scalar.copy(st["cidRow"][0:1, :], pcid[0:1, :])
                nc.vector.memset(st["mcRec"][:, :], 0.0)
                nc.vector.memset(st["ralC"][:, :], 1.0)
                nc.vector.memset(st["calC"][:, :], 1.0)

            def tail_round_t1(st, r):
                rmC, cmC = st["rmC"], st["cmC"]
                acC, arC = st["acC"], st["arC"]
                u1, u2, u3, u4 = st["u1"], st["u2"], st["u3"], st["u4"]
                if r > 0:
                    nc.gpsimd.tensor_tensor(out=st["Wc"][:, :],
                                            in0=st["Wc"][:, :],
                                            in1=st["alvBC"][:, 0:128],
                                            op=AL.mult)
                nc.vector.max(st["m8c"][:, :], st["Wc"][:, :])
                nc.vector.max_index(st["i8c"][:, :], st["m8c"][:, :],
                                    st["Wc"][:, :])
                nc.scalar.copy(rmC[:, 0:1], st["m8c"][:, 0:1])
                nc.scalar.copy(acC[:, 0:1], st["i8c"][:, 0:1])
                if r > 0:
                    nc.vector.tensor_tensor(out=st["WtC"][:, :],
                                            in0=st["WtC"][:, :],
                                            in1=st["alvBC"][:, 128:256],
                                            op=AL.mult)
                nc.vector.max(st["m8d"][:, :], st["WtC"][:, :])
                nc.vector.max_index(st["i8d"][:, :], st["m8d"][:, :],
                                    st["WtC"][:, :])
                nc.scalar.copy(cmC[:, 0:1], st["m8d"][:, 0:1])
                nc.scalar.copy(arC[:, 0:1], st["i8d"][:, 0:1])
                rkC, ckC = st["rkC"], st["ckC"]
                nc.vector.scalar_tensor_tensor(out=u1[:, :], in0=acC[:, :],
                                               scalar=2.0, in1=iotaRowKeyC,
                                               op0=AL.add, op1=AL.add)
                nc.vector.scalar_tensor_tensor(out=u2[:, :], in0=rmC[:, :],
                                               scalar=0.0,
                                               in1=st["ralC"][:, :],
                                               op0=AL.is_gt, op1=AL.mult)
                nc.vector.tensor_tensor(out=rkC[:, :], in0=u1[:, :],
                                        in1=u2[:, :], op=AL.mult)
                nc.vector.tensor_scalar(out=u3[:, :], in0=arC[:, :],
                                        scalar1=128.0, scalar2=2.0,
                                        op0=AL.mult, op1=AL.add)
                nc.vector.tensor_tensor(out=u3[:, :], in0=u3[:, :],
                                        in1=iotaP, op=AL.add)
                nc.vector.scalar_tensor_tensor(out=u4[:, :], in0=cmC[:, :],
                                               scalar=0.0,
                                               in1=st["calC"][:, :],
                                               op0=AL.is_gt, op1=AL.mult)
                nc.vector.tensor_tensor(out=ckC[:, :], in0=u3[:, :],
                                        in1=u4[:, :], op=AL.mult)
                bcast128x2(ckC, rkC, st["keyRowC"], st["keyBC"])

            def tail_round_t2(st, r):
                scrC, scrC2 = st["scrC"], st["scrC2"]
                acC = st["acC"]
                # matched-ts dummy outs use scrC/scrC2 (free now)
                rkC, ckC = st["rkC"], st["ckC"]
                u1, u2, u3, u4 = st["u1"], st["u2"], st["u3"], st["u4"]
                mrC, mcC = st["mrC"], st["mcC"]
                nc.vector.tensor_scalar(
                    out=scrC2[:, :], in0=st["keyBC"][:, 0:128],
                    scalar1=rkC[:, 0:1],
                    scalar2=0.0, op0=AL.is_equal, op1=AL.max,
                    accum_out=mrC[:, 0:1])
                nc.vector.tensor_scalar(
                    out=scrC[:, :], in0=st["keyBC"][:, 128:256],
                    scalar1=ckC[:, 0:1],
                    scalar2=0.0, op0=AL.is_equal, op1=AL.max,
                    accum_out=mcC[:, 0:1])
                nc.vector.tensor_tensor(out=mrC[:, :], in0=mrC[:, :],
                                        in1=u2[:, :], op=AL.mult)
                nc.vector.tensor_tensor(out=mcC[:, :], in0=mcC[:, :],
                                        in1=u4[:, :], op=AL.mult)
                nc.vector.tensor_scalar(out=u1[:, :], in0=acC[:, :],
                                        scalar1=1.0, scalar2=None, op0=AL.add)
                nc.vector.tensor_tensor(out=u1[:, :], in0=u1[:, :],
                                        in1=mrC[:, :], op=AL.mult)
                nc.vector.tensor_tensor(out=st["mcRec"][:, :],
                                        in0=st["mcRec"][:, :],
                                        in1=u1[:, :], op=AL.max)
                nc.vector.scalar_tensor_tensor(out=st["ralC"][:, :],
                                               in0=mrC[:, :], scalar=-1.0,
                                               in1=st["ralC"][:, :],
                                               op0=AL.mult, op1=AL.add)
                nc.vector.scalar_tensor_tensor(out=st["calC"][:, :],
                                               in0=mcC[:, :], scalar=-1.0,
                                               in1=st["calC"][:, :],
                                               op0=AL.mult, op1=AL.add)
                if r + 1 < tail_rounds:
                    bcast128x2(st["calC"], st["ralC"], st["alvRowC"],
                               st["alvBC"])

            def output(st, m):
                # orig col of tail matches: onehot(mcRec-1) . cid
                mm1, mo, gt0 = st["u1"], st["u2"], st["u3"]
                nc.vector.tensor_scalar(out=mm1[:, :], in0=st["mcRec"][:, :],
                                        scalar1=-1.0, scalar2=None, op0=AL.add)
                Omc = st["scrC"]
                nc.vector.tensor_scalar(out=Omc[:, :], in0=iotaF128,
                                        scalar1=mm1[:, 0:1], scalar2=None,
                                        op0=AL.is_equal)
                nc.gpsimd.partition_broadcast(st["cidB"][:, :],
                                              st["cidRow"][0:1, :])
                nc.vector.tensor_tensor(out=Omc[:, :], in0=Omc[:, :],
                                        in1=st["cidB"][:, :], op=AL.mult)
                nc.vector.tensor_reduce(out=mo[:, 0:1], in_=Omc[:, :],
                                        axis=AX.X, op=AL.add)
                nc.vector.tensor_scalar(out=gt0[:, :], in0=st["mcRec"][:, :],
                                        scalar1=0.0, scalar2=None, op0=AL.is_gt)
                nc.vector.tensor_scalar(out=mo[:, :], in0=mo[:, :],
                                        scalar1=1.0, scalar2=None, op0=AL.add)
                nc.vector.tensor_tensor(out=mo[:, :], in0=mo[:, :],
                                        in1=gt0[:, :], op=AL.mult)
                pmb = pool_ps.tile([128, 4], FP, name="ps", tag="ps")
                for k in range(4):
                    Gr = st["scrC2"]
                    nc.vector.tensor_scalar(out=st["u4"][:, :],
                                            in0=st["rid"][:, :],
                                            scalar1=float(-128 * k),
                                            scalar2=None, op0=AL.add)
                    nc.vector.tensor_scalar(out=Gr[:, :], in0=iotaF128,
                                            scalar1=st["u4"][:, 0:1],
                                            scalar2=None, op0=AL.is_equal)
                    nc.tensor.matmul(pmb[:, k:k + 1], Gr[:, :], mo[:, 0:1],
                                     start=True, stop=True)
                mcb = st["t2"]
                nc.vector.tensor_copy(mcb[:, :], pmb[:, :])
                nc.vector.tensor_tensor(out=st["mc"][:, :], in0=st["mc"][:, :],
                                        in1=mcb[:, :], op=AL.max)
                # ---- L1 matches: mc1 [128,2] (L1 col id +1) -> orig space
                cid1B = st["cid1Bs"][:, :]
                trash = st["trash"]
                for b in range(2):
                    nc.vector.tensor_scalar(out=st["g1"][:, 0:1],
                                            in0=st["mc1"][:, b:b + 1],
                                            scalar1=-1.0, scalar2=None,
                                            op0=AL.add)
                    nc.vector.tensor_scalar(out=trash[:, 0:160], in0=iotaF160,
                                            scalar1=st["g1"][:, 0:1],
                                            scalar2=None, op0=AL.is_equal)
                    nc.vector.tensor_tensor(out=trash[:, 256:416],
                                            in0=trash[:, 0:160],
                                            in1=cid1B, op=AL.mult)
                    nc.vector.tensor_reduce(out=st["mo1"][:, b:b + 1],
                                            in_=trash[:, 256:416],
                                            axis=AX.X, op=AL.add)
                    nc.vector.tensor_scalar(out=st["g1"][:, 1:2],
                                            in0=st["mc1"][:, b:b + 1],
                                            scalar1=0.0, scalar2=None,
                                            op0=AL.is_gt)
                    nc.vector.tensor_scalar(out=st["mo1"][:, b:b + 1],
                                            in0=st["mo1"][:, b:b + 1],
                                            scalar1=1.0, scalar2=None,
                                            op0=AL.add)
                    nc.vector.tensor_tensor(out=st["mo1"][:, b:b + 1],
                                            in0=st["mo1"][:, b:b + 1],
                                            in1=st["g1"][:, 1:2], op=AL.mult)
                pm1 = pool_ps.tile([128, 4], FP, name="ps", tag="ps")
                for k in range(4):
                    for b in range(2):
                        win = trash[:, 0:128] if b == 0 else trash[:, 128:256]
                        nc.vector.tensor_scalar(out=st["u4"][:, :],
                                                in0=st["rid1p"][:, b:b + 1],
                                                scalar1=float(-128 * k),
                                                scalar2=None, op0=AL.add)
                        nc.vector.tensor_scalar(out=win, in0=iotaF128,
                                                scalar1=st["u4"][:, 0:1],
                                                scalar2=None, op0=AL.is_equal)
                        nc.tensor.matmul(pm1[:, k:k + 1], win,
                                         st["mo1"][:, b:b + 1],
                                         start=(b == 0), stop=(b == 1))
                nc.vector.tensor_copy(mcb[:, :], pm1[:, :])
                nc.vector.tensor_tensor(out=st["mc"][:, :], in0=st["mc"][:, :],
                                        in1=mcb[:, :], op=AL.max)
                s4 = st["t4"]
                nc.vector.tensor_scalar(out=s4[:, :], in0=st["mc"][:, :],
                                        scalar1=-1.0, scalar2=513.0,
                                        op0=AL.mult, op1=AL.add)
                for k in range(4):
                    ot = pool_out.tile([128, 512], FP, name=f"ot{k % 2}",
                                       tag=f"ot{k % 2}")
                    nc.vector.tensor_scalar(out=ot[:, :], in0=iotaDesc,
                                            scalar1=s4[:, k:k + 1],
                                            scalar2=None, op0=AL.is_equal)
                    nc.sync.dma_start(out=out_ap[m, 128 * k:128 * (k + 1), :],
                                      in_=ot[:, :])

            # ================= interleaved emission =================
            mat_list = list(range(n_mat)) * repeat
            G0 = min(group, len(mat_list))
            for s in range(G0):
                load(states[s], mat_list[s])
            for g0 in range(0, len(mat_list), group):
                G = min(group, len(mat_list) - g0)
                for s in range(G):
                    init_vecs(states[s])
                for r in range(full_rounds):
                    for s in range(G):
                        full_round_h1(states[s], r)
                    for s in range(G):
                        full_round_h2(states[s], r)
                for s in range(G):
                    compact1(states[s])
                for r in range(l1_rounds):
                    for s in range(G):
                        l1_h1(states[s], r)
                    for s in range(G):
                        l1_h2(states[s], r, l1_rounds)
                for s in range(G):
                    compact2(states[s])
                nxt = g0 + group
                if nxt < len(mat_list):
                    for s in range(min(group, len(mat_list) - nxt)):
                        load(states[s], mat_list[nxt + s])
                for r in range(tail_rounds):
                    for s in range(G):
                        tail_round_t1(states[s], r)
                    for s in range(G):
                        tail_round_t2(states[s], r)
                for s in range(G):
                    output(states[s], mat_list[g0 + s])
    return nc



# ----------------------------------------------------------------------------
# Host-side entry point: shard the 256-matrix batch over 8 NeuronCores
# (pure data parallelism, 32 matrices per core), run the SPMD kernel,
# reassemble, and exactly recompute any matrix whose output fails the
# permutation sum check (defence in depth; does not trigger on the
# reference input -- tie-breaking on device matches jnp.argmax exactly).
# ----------------------------------------------------------------------------
from concourse.bass_utils import run_bass_kernel_spmd

N_CORES = 8
B, N = 256, 512
MPC = B // N_CORES  # matrices per core


def _greedy_ref_one(w):
    """Exact numpy mirror of the jax reference for one [N,N] matrix."""
    w = w.copy()
    perm = np.zeros_like(w)
    for _ in range(N):
        flat = np.argmax(w)
        r, c = flat // N, flat % N
        perm[r, c] = 1.0
        w[r, :] = 0.0
        w[:, c] = 0.0
    return perm


_CACHE = {}


def _get_graph():
    if "nc" not in _CACHE:
        nc = bacc.Bacc()
        s_ext = nc.declare_dram_parameter("s", [MPC, N, N], FP, isOutput=False)
        c_ext = nc.declare_dram_parameter("consts", [128, CONST_W], FP,
                                          isOutput=False)
        o_ext = nc.declare_dram_parameter("out", [MPC, N, N], FP, isOutput=True)
        build_nms_kernel(nc, o_ext, s_ext, c_ext, n_mat=MPC)
        nc.finalize()
        _CACHE["nc"] = nc
    return _CACHE["nc"]


def kernel(s: np.ndarray) -> np.ndarray:
    s = np.ascontiguousarray(np.asarray(s), dtype=np.float32)
    assert s.shape == (B, N, N)
    nc = _get_graph()
    consts = make_consts()
    shards = s.reshape(N_CORES, MPC, N, N)
    in_maps = [{"s": shards[i], "consts": consts} for i in range(N_CORES)]
    res = run_bass_kernel_spmd(nc, in_maps, core_ids=list(range(N_CORES)))
    out = np.concatenate([np.asarray(res.results[i]["out"])
                          for i in range(N_CORES)], axis=0)
    out = out.reshape(B, N, N).astype(np.float32)
    # safety net: matrices the on-device rounds did not fully converge on
    # (a handful of stragglers with <=5 rows left) are completed exactly by
    # continuing the greedy matching on the tiny residual submatrix; any
    # structurally corrupt perm falls back to a full exact recompute.
    rs = out.sum(axis=2)
    cs = out.sum(axis=1)
    check = np.where((rs != 1.0).any(axis=1) | (cs != 1.0).any(axis=1))[0]
    for b in check:
        if (rs[b] > 1.0).any() or (cs[b] > 1.0).any():
            out[b] = _greedy_ref_one(s[b])
            continue
        ur = np.where(rs[b] == 0.0)[0]
        uc = np.where(cs[b] == 0.0)[0]
        if len(ur) != len(uc):
            out[b] = _greedy_ref_one(s[b])
            continue
        # continue greedy on the residual submatrix (exact: matched pairs of
        # locally-dominant rounds are a prefix-closed subset of greedy's)
        sub = s[b][np.ix_(ur, uc)].copy()
        n = len(ur)
        for _ in range(n):
            flat = np.argmax(sub)
            r, c = flat // n, flat % n
            out[b, ur[r], uc[c]] = 1.0
            sub[r, :] = 0.0
            sub[:, c] = 0.0
    return out

